# revision 30
# baseline (speedup 1.0000x reference)
"""Causal self-attention with LoRA (q,v) — Trainium2 Bass kernel, 8 cores.

Sharding: data-parallel over batch (B=2), tensor-parallel over heads
(16 heads -> 4 per core).  Core c handles batch c//4, heads 4*(c%4)..+4.
Each core computes its 256-dim q/k/v projection slice from the full
hidden states and its heads' full 2048x2048 causal attention locally.
No collectives; host does the (layout-only) scatter/gather + fp16 casts
+ sbuf-image tiling so every bulk load is one contiguous HWDGE DMA.

The whole datapath runs in fp16 with fp32 PSUM accumulation (measured
rel err vs the fp32 reference ~5e-4).  fp16 matters a lot on TRN2: f32r
matmuls cannot use standalone LDWEIGHTS (walrus bug) so every f32r MM
pays an inline ~214ns weight load, and FP32_HIGH disables fast weight
load entirely; fp16 streams at 1 col/cycle with LDW hidden by the PE
reorder window.

Attention is computed in transposed orientation (scores sT[j, i]) so no
on-chip transposes are needed; the softmax denominator rides along the
PV matmul as a 65th lhsT column (augmented-V), which also folds the
additive attention mask in exactly (em = exp(mask) scaling of V rows).
Score psum groups pack 3 head-chunks (1536 cols) so the ACT exp — the
throughput limiter of the attention phase at 1 elem/lane/cycle — pays
its ~352-cycle per-instruction overhead a third as often.  Causal
masking is a multiplicative staircase applied after exp; the two
most-diagonal 128-key chunks per 512-query block are cropped to their
upper 256 columns (laid out so concurrently issued row-packed matmul
pairs never write the same PSUM bank).  Scores for the two heads of a
pair run concurrently via K=64 row packing.  Emission follows x-quarter
arrival so the first attention block starts ~10us in.
"""

import numpy as np

B, T, DM, H = 2, 2048, 1024, 16
HD = 64          # head dim
R = 8            # LoRA rank
NCORES = 8
GPB = 4          # head-groups (cores) per batch
HPC = 4          # heads per core
DPC = HPC * HD   # 256 output dims per core
LORA_SCALE = 2.0  # lora_alpha / r
SM_SCALE = HD ** -0.5  # 0.125

KC = DM // 128   # 8 contraction chunks
MC = DPC // 128  # 2 output-dim chunks (head pairs)
NB = 4           # t-blocks (x quarters) for q/k projections
TCH = T // 128   # 16 t-chunks (key chunks)
IBN = T // 512   # 4 query i-blocks (512 wide)
GCH = 2          # score-group capacity in 512-wide chunk units

_CACHE = {}


def _build_program():
    from contextlib import ExitStack

    import concourse.bass as bass
    import concourse.tile as tile
    from concourse import bacc, mybir

    f32 = mybir.dt.float32
    f16 = mybir.dt.float16
    EXP = mybir.ActivationFunctionType.Exp
    ts = bass.ts

    nc = bacc.Bacc(
        "TRN2",
        target_bir_lowering=False,
        debug=False,
        enable_asserts=True,
        num_devices=NCORES,
    )

    xq = nc.dram_tensor("xq", [NB, 128, KC * 512], f16, kind="ExternalInput").ap()
    wq_img = nc.dram_tensor("wq_img", [128, KC * DPC], f16, kind="ExternalInput").ap()
    wk_img = nc.dram_tensor("wk_img", [128, KC * DPC], f16, kind="ExternalInput").ap()
    wv_img = nc.dram_tensor("wv_img", [128, KC * DPC], f16, kind="ExternalInput").ap()
    a_both = nc.dram_tensor("a_both", [R, 2 * DM], f16, kind="ExternalInput").ap()
    bT_both = nc.dram_tensor("bT_both", [R, 2 * DPC], f16, kind="ExternalInput").ap()
    biasqk = nc.dram_tensor("biasqk", [128, 4], f32, kind="ExternalInput").ap()
    biasv = nc.dram_tensor("biasv", [DPC], f16, kind="ExternalInput").ap()
    amask = nc.dram_tensor("amask", [T], f32, kind="ExternalInput").ap()
    outT = nc.dram_tensor("outT", [DPC, T], f32, kind="ExternalOutput").ap()

    with tile.TileContext(nc) as tc, ExitStack() as ctx:
        const = ctx.enter_context(tc.tile_pool(name="const", bufs=1))
        xpool = ctx.enter_context(tc.tile_pool(name="x", bufs=1))
        wpool = ctx.enter_context(tc.tile_pool(name="w", bufs=1))
        qkpool = ctx.enter_context(tc.tile_pool(name="qk", bufs=1))
        vpool = ctx.enter_context(tc.tile_pool(name="v", bufs=1))
        ppool = ctx.enter_context(tc.tile_pool(name="pT", bufs=4))
        opool = ctx.enter_context(tc.tile_pool(name="osb", bufs=4))
        psum = ctx.enter_context(tc.tile_pool(name="psum", bufs=3, space="PSUM"))
        popool = ctx.enter_context(tc.tile_pool(name="po", bufs=1, space="PSUM"))

        def stair_slice(d, qo, w):
            # chunk with diagonal offset d cropped to queries [qo, qo+w):
            # valid iff j >= p + 128 d - qo; stair[p, s+j] = (j >= p) at
            # s = 384, shifted by the residue 128 d - qo (0 for every
            # crop in use, since qo = 128 d).
            start = 384 - 128 * d + qo
            return stair[:, start : start + w]

        # ---------------- loads, critical-path first ------------------
        # DMA engines drain queue descriptors roughly FIFO across
        # queues, so the small LoRA/weight transfers the first matmuls
        # depend on are issued before the 4MB of x.
        # scalar queue: fold operands + x; sync queue: weight images +
        # small consts — the two queues drain in parallel so the
        # fold-matmul inputs and first x quarter land while wq streams.
        a_sb = const.tile([R, 2 * DM], f16, tag="a")
        nc.scalar.dma_start(a_sb[:], a_both)
        bT_sb = const.tile([R, 2 * DPC], f16, tag="bT")
        nc.scalar.dma_start(bT_sb[:], bT_both)
        wq_all = wpool.tile([128, KC * DPC], f16, tag="wq_all")
        nc.sync.dma_start(wq_all[:], wq_img)
        wk_all = wpool.tile([128, KC * DPC], f16, tag="wk_all")
        nc.sync.dma_start(wk_all[:], wk_img)

        xall = xpool.tile([128, KC * T], f16, tag="xall")
        x3d = xall[:].rearrange("p (kc c) -> p kc c", kc=KC)
        nc.scalar.dma_start(x3d[:, :, ts(0, 512)], xq[0])

        wv_all = wpool.tile([128, KC * DPC], f16, tag="wv_all")
        nc.sync.dma_start(wv_all[:], wv_img)
        bias_sb = const.tile([128, 4], f32, tag="biasqk")
        nc.sync.dma_start(bias_sb[:], biasqk)
        bv_row = const.tile([1, DPC], f16, tag="bvrow")
        nc.sync.dma_start(bv_row[:], biasv.unsqueeze(0))
        em_raw = const.tile([128, TCH], f32, tag="em_raw")
        nc.sync.dma_start(em_raw[:], amask.rearrange("(c p) -> p c", p=128))

        for q in range(1, NB):
            nc.scalar.dma_start(x3d[:, :, ts(q, 512)], xq[q])

        def xs(kc, lo, n):
            return xall[:, 2048 * kc + lo : 2048 * kc + lo + n]

        ones_1xP = const.tile([1, 128], f16, tag="ones")
        nc.vector.memset(ones_1xP[:], 1.0)

        # dummy matmuls fill the PE while the first DMAs land, warming
        # the HAM clock-gate (first ~3.4us of PE activity runs at 1.2
        # instead of 2.4 GHz) so the real stream starts at full clock
        for _ in range(28):
            wps = psum.tile([128, 128], f32, tag="sc")
            nc.tensor.matmul(wps[:], ones_1xP[:], ones_1xP[:], start=True, stop=True)

        # em[p, jb] = exp(amask[128*jb + p])
        em = const.tile([128, TCH], f32, tag="em")
        nc.scalar.activation(em[:], em_raw[:], EXP)

        # Causal staircase (multiplicative, applied after exp):
        # stair[p, m] = 1.0 if m >= p + 384 else 0.0 ; shape [128, 896].
        stair = const.tile([128, 896], f16, tag="stair")
        nc.gpsimd.memset(stair[:], 1.0)
        nc.gpsimd.affine_select(
            out=stair[:],
            in_=stair[:],
            compare_op=mybir.AluOpType.is_ge,
            fill=0.0,
            base=-384,
            pattern=[[1, 896]],
            channel_multiplier=-1,
        )

        # ---------------- weights with LoRA fold for q, v ---------------
        def load_folded(w_all, a_off, bT_off, name):
            """W'.T = W.T + A.T @ (2 B.T), one [128, KC*DPC] tile."""
            wf = wpool.tile([128, KC * DPC], f16, tag=f"wf_{name}")
            for kc in range(KC):
                dps = psum.tile([128, DPC], f32, tag="sc")
                nc.tensor.matmul(
                    dps[:],
                    a_sb[:, a_off + 128 * kc : a_off + 128 * kc + 128],
                    bT_sb[:, bT_off : bT_off + DPC],
                    start=True,
                    stop=True,
                )
                nc.vector.tensor_add(
                    wf[:, ts(kc, DPC)], w_all[:, ts(kc, DPC)], dps[:]
                )
            return wf

        wq_f = load_folded(wq_all, 0, 0, "q")

        # ---------------- projections ----------------
        # qT/kT: [d, t] with d on partitions; tile mc holds head pair
        # (2mc, 2mc+1): partitions 0-63 = head 2mc, 64-127 = head 2mc+1.
        qT_sb = [qkpool.tile([128, T], f16, tag=f"qT{mc}", name=f"qT{mc}") for mc in range(MC)]
        kT_sb = [qkpool.tile([128, T], f16, tag=f"kT{mc}", name=f"kT{mc}") for mc in range(MC)]

        def project_qk(wf, dst, bias, mc, nb):
            ps = psum.tile([128, 512], f32, tag="sc")
            for kc in range(KC):
                nc.tensor.matmul(
                    ps[:],
                    wf[:, kc * DPC + mc * 128 : kc * DPC + mc * 128 + 128],
                    xs(kc, 512 * nb, 512),
                    start=(kc == 0),
                    stop=(kc == KC - 1),
                )
            nc.vector.tensor_add(
                dst[:, ts(nb, 512)],
                ps[:],
                bias.to_broadcast((128, 512)),
            )

        # v in natural [t, d] orientation, em-scaled, with the denominator
        # (em) column appended per head: [128, 4*65].
        v2_sb = [vpool.tile([128, HPC * (HD + 1)], f16, tag=f"v2{j}", name=f"v2{j}") for j in range(TCH)]

        def project_v(jbs):
            for jb in jbs:
                ps = psum.tile([128, DPC], f32, tag="sc")
                for kc in range(KC):
                    nc.tensor.matmul(
                        ps[:],
                        xs(kc, 128 * jb, 128),
                        wv_f[:, ts(kc, DPC)],
                        start=(kc == 0),
                        stop=False,
                    )
                nc.tensor.matmul(  # + ones(t) x bias_v
                    ps[:],
                    ones_1xP[:],
                    bv_row[:],
                    start=False,
                    stop=True,
                )
                v2 = v2_sb[jb]
                em_col = em[:, jb : jb + 1]
                nc.vector.tensor_mul(
                    v2[:].rearrange("p (h c) -> p h c", h=HPC)[:, :, 0:HD],
                    ps[:].rearrange("p (h c) -> p h c", h=HPC),
                    em_col.unsqueeze(1).broadcast_to((128, HPC, HD)),
                )
                nc.vector.tensor_copy(
                    v2[:, HD : HPC * (HD + 1) : HD + 1],
                    em_col.to_broadcast((128, HPC)),
                )

        # ---------------- attention for one head pair ----------------
        def attention_ib(pr, ib):
            qT, kT = qT_sb[pr], kT_sb[pr]
            nch = 4 * ib + 4  # causal key chunks per head
            # chunk stream: (hl, jb, qoff, width, d); d = diagonal
            # offset.  Diagonal chunks are cropped to exactly the
            # causally reachable query range: d=1 -> 384@128, d=2 ->
            # 256@256, d=3 -> 128@384 (d=0 stays full so the first PV
            # write covers the whole po region).
            fulls = []
            for jb in range(4 * ib + 1):
                d = jb - 4 * ib
                for hl in (0, 1):
                    fulls.append((hl, jb, 0, 512, d))
            jd = 4 * ib
            # groups: list of (chunk, col_off).  Fulls are packed GCH
            # chunks to a psum group (column offsets 0/512) so one ACT
            # exp instruction covers up to 1024 columns.  Adjacent
            # offsets always land in different psum banks, so the
            # concurrently-running row-packed (h, h') score pairs never
            # write the same bank.  Diagonal cluster: group B holds the
            # d=2 pair (h@0, h'@512, strided exp skips the holes);
            # group A packs d=1 (384@0/512) and d=3 (128@384/896)
            # hole-free.  B is emitted before A so the d=3 chunk — the
            # accumulation-group stop — is the last PV.
            groups = []
            for i in range(0, len(fulls), GCH):
                groups.append([(c, j * 512) for j, c in enumerate(fulls[i : i + GCH])])
            groups.append([
                ((0, jd + 2, 256, 256, 2), 0),
                ((1, jd + 2, 256, 256, 2), 512),
            ])
            groups.append([
                ((0, jd + 1, 128, 384, 1), 0),
                ((1, jd + 1, 128, 384, 1), 512),
                ((0, jd + 3, 384, 128, 3), 384),
                ((1, jd + 3, 384, 128, 3), 896),
            ])

            po = [
                popool.tile([65, 512], f32, tag=f"po{hl}", name=f"po{pr}_{ib}_{hl}")
                for hl in (0, 1)
            ]
            for g in groups:
                extent = 512 if len(g) == 1 else 1024
                covered = sum(c[3] for c, _ in g)
                ps = psum.tile([128, extent], f32, tag="sc")
                for (hl, jb, qo, w, d), off in g:
                    nc.tensor.matmul(
                        ps[:, off : off + w],
                        kT[ts(hl, 64), ts(jb, 128)],
                        qT[ts(hl, 64), ib * 512 + qo : ib * 512 + qo + w],
                        start=True,
                        stop=True,
                    )
                pT = ppool.tile([128, extent], f16, tag="pT")
                if covered == extent:
                    nc.scalar.activation(pT[:], ps[:], EXP, scale=SM_SCALE)
                else:
                    # holed group (d=2 pair at 0/512): strided exp skips
                    # the stale psum columns (exp there could overflow)
                    w0 = g[0][0][3]
                    nc.scalar.activation(
                        pT[:].rearrange("p (q h) -> p q h", q=2)[:, :, 0:w0],
                        ps[:].rearrange("p (q h) -> p q h", q=2)[:, :, 0:w0],
                        EXP,
                        scale=SM_SCALE,
                    )
                # causal staircase on partial chunks; merge the
                # (h, h') twin segments (always 512 apart) into one 3-D op
                i = 0
                while i < len(g):
                    (hl, jb, qo, w, d), off_i = g[i]
                    if d < 0:
                        i += 1
                        continue
                    msk = stair_slice(d, qo, w)
                    twin = (
                        i + 1 < len(g)
                        and g[i + 1][0][1] == jb
                        and g[i + 1][0][3] == w
                        and g[i + 1][1] == off_i + 512
                    )
                    if twin:
                        seg = pT[:].rearrange("p (q h) -> p q h", q=2)[
                            :, :, off_i : off_i + w
                        ]
                        nc.vector.tensor_mul(
                            seg,
                            seg,
                            msk.unsqueeze(1).broadcast_to((128, 2, w)),
                        )
                        i += 2
                    else:
                        nc.vector.tensor_mul(
                            pT[:, off_i : off_i + w],
                            pT[:, off_i : off_i + w],
                            msk,
                        )
                        i += 1
                # PV: outT[d, i] accumulation per head; denominator
                # column (em) rides along as lhsT column 64.
                for (hl, jb, qo, w, d), off in g:
                    nc.tensor.matmul(
                        po[hl][:, qo : qo + w],
                        v2_sb[jb][:, (2 * pr + hl) * (HD + 1) : (2 * pr + hl + 1) * (HD + 1)],
                        pT[:, off : off + w],
                        start=(jb == 0),
                        stop=(jb == nch - 1),
                    )
            # normalize: out[:64] / denom (row 64), per column — both
            # heads batched through one reshape/reciprocal/broadcast
            # chain to halve the serial tail latency.
            sbp = opool.tile([65, 1024], f32, tag="sbp")
            # denominator rows first so the reshape DMA launches before
            # the bulk copies run
            for hl in (0, 1):
                nc.vector.tensor_copy(sbp[64:65, ts(hl, 512)], po[hl][64:65, :])
            # reshape the denominator row onto 128 partitions so the
            # (slow, free-size-bound) DVE reciprocal runs on [128, 8]
            dn = opool.tile([128, 8], f32, tag="dn")
            nc.scalar.dma_start(
                dn[:], sbp[64:65, :].rearrange("o (p c) -> o p c", p=128)
            )
            for hl in (0, 1):
                nc.vector.tensor_copy(sbp[0:64, ts(hl, 512)], po[hl][0:64, :])
            dnr = opool.tile([128, 8], f32, tag="dnr")
            nc.vector.reciprocal(dnr[:], dn[:])
            rc = opool.tile([1, 1024], f32, tag="rc")
            nc.sync.dma_start(
                rc[:].rearrange("o (p c) -> o p c", p=128), dnr[:]
            )
            # broadcast/multiply/store per head so the first head's
            # output DMA overlaps the second head's broadcast
            rb = opool.tile([64, 1024], f32, tag="rb")
            oT = opool.tile([64, 1024], f32, tag="oT")
            for hl in (0, 1):
                nc.gpsimd.partition_broadcast(rb[:, ts(hl, 512)], rc[:, ts(hl, 512)])
                nc.vector.tensor_mul(
                    oT[:, ts(hl, 512)], sbp[0:64, ts(hl, 512)], rb[:, ts(hl, 512)]
                )
                nc.sync.dma_start(
                    outT[(2 * pr + hl) * HD : (2 * pr + hl + 1) * HD, ts(ib, 512)],
                    oT[:, ts(hl, 512)],
                )

        # emission order follows x-quarter arrival: per quarter, pair-0
        # projections + attention, then pair-1 likewise — projections
        # act as PE filler while ACT exps the other pair's scores.  The
        # wv fold is emitted after the first q/k projections so the PE
        # queue doesn't head-of-line block on the wv DMA.
        # Per quarter: ALL projections first (their DVE bias-adds and v2
        # ops precede the attention masks in the in-order DVE queue, so
        # weight production is never head-of-line blocked behind masks),
        # then the two attention calls.
        project_qk(wq_f, qT_sb[0], bias_sb[:, 0:1], 0, 0)
        project_qk(wk_all, kT_sb[0], bias_sb[:, 2:3], 0, 0)
        wv_f = load_folded(wv_all, DM, DPC, "v")
        for nb in range(NB):
            if nb > 0:
                project_qk(wq_f, qT_sb[0], bias_sb[:, 0:1], 0, nb)
                project_qk(wk_all, kT_sb[0], bias_sb[:, 2:3], 0, nb)
            project_v(range(4 * nb, 4 * nb + 4))
            project_qk(wq_f, qT_sb[1], bias_sb[:, 1:2], 1, nb)
            project_qk(wk_all, kT_sb[1], bias_sb[:, 3:4], 1, nb)
            attention_ib(0, nb)
            attention_ib(1, nb)

    nc.compile()
    return nc


def _shard_inputs(inputs):
    """Full inputs -> per-core input maps (host-side layout + fp16 cast)."""
    f16 = np.float16
    hs = np.asarray(inputs["hidden_states"], dtype=np.float32)
    am = np.asarray(inputs["attention_mask"], dtype=np.float32)
    Wq = np.asarray(inputs["Wq"], dtype=np.float32)
    Wk = np.asarray(inputs["Wk"], dtype=np.float32)
    Wv = np.asarray(inputs["Wv"], dtype=np.float32)
    bq = np.asarray(inputs["bq"], dtype=np.float32)
    bk = np.asarray(inputs["bk"], dtype=np.float32)
    bv = np.asarray(inputs["bv"], dtype=np.float32)
    Aq = np.asarray(inputs["Aq"], dtype=np.float32)
    Bq = np.asarray(inputs["Bq"], dtype=np.float32)
    Av = np.asarray(inputs["Av"], dtype=np.float32)
    Bv = np.asarray(inputs["Bv"], dtype=np.float32)

    c = np.ascontiguousarray

    def wimg(W, sl):
        # sbuf image: wimg[p, 256*kc + j] = W[sl].T[128*kc + p, j]
        return c(W[sl].T.astype(f16).reshape(KC, 128, DPC).transpose(1, 0, 2).reshape(128, KC * DPC))

    # x quarter images: Xq[q, p, 512*kc + cc] = hs[b].T[128*kc + p, 512*q + cc]
    xqs = [
        c(hs[b].T.astype(f16).reshape(KC, 128, NB, 512).transpose(2, 1, 0, 3).reshape(NB, 128, KC * 512))
        for b in range(B)
    ]
    a_both = c(np.concatenate([Aq, Av], axis=1).astype(f16))
    in_maps = []
    for core in range(NCORES):
        b, g = core // GPB, core % GPB
        sl = slice(g * DPC, (g + 1) * DPC)
        bqk = np.stack(
            [bq[sl][:128], bq[sl][128:], bk[sl][:128], bk[sl][128:]], axis=1
        )
        bT = np.concatenate(
            [LORA_SCALE * Bq[sl].T, LORA_SCALE * Bv[sl].T], axis=1
        ).astype(f16)
        in_maps.append(
            {
                "xq": xqs[b],
                "wq_img": wimg(Wq, sl),
                "wk_img": wimg(Wk, sl),
                "wv_img": wimg(Wv, sl),
                "a_both": a_both,
                "bT_both": c(bT),
                "biasqk": c(bqk),
                "biasv": c(bv[sl].astype(f16)),
                "amask": c(am[b, 0, 0, :]),
            }
        )
    return in_maps


def _run(inputs, trace=False):
    from concourse.bass_utils import run_bass_kernel_spmd

    if "nc" not in _CACHE:
        _CACHE["nc"] = _build_program()
    nc = _CACHE["nc"]
    in_maps = _shard_inputs(inputs)
    res = run_bass_kernel_spmd(nc, in_maps, list(range(NCORES)), trace=trace)
    out = np.empty((B, T, DM), dtype=np.float32)
    for core in range(NCORES):
        b, g = core // GPB, core % GPB
        out[b, :, g * DPC : (g + 1) * DPC] = res.results[core]["outT"].T
    return out, res


def kernel(**inputs) -> np.ndarray:
    out, _ = _run(inputs, trace=False)
    return out


# revision 33
# speedup vs baseline: 1.0299x; 1.0299x over previous
"""Causal self-attention with LoRA (q,v) — Trainium2 Bass kernel, 8 cores.

Sharding: data-parallel over batch (B=2), tensor-parallel over heads
(16 heads -> 4 per core).  Core c handles batch c//4, heads 4*(c%4)..+4.
Each core computes its 256-dim q/k/v projection slice from the full
hidden states and its heads' full 2048x2048 causal attention locally.
No collectives; host does the (layout-only) scatter/gather + fp16 casts
+ sbuf-image tiling so every bulk load is one contiguous HWDGE DMA.

The whole datapath runs in fp16 with fp32 PSUM accumulation (measured
rel err vs the fp32 reference ~5e-4).  fp16 matters a lot on TRN2: f32r
matmuls cannot use standalone LDWEIGHTS (walrus bug) so every f32r MM
pays an inline ~214ns weight load, and FP32_HIGH disables fast weight
load entirely; fp16 streams at 1 col/cycle with LDW hidden by the PE
reorder window.

Attention is computed in transposed orientation (scores sT[j, i]) so no
on-chip transposes are needed; the softmax denominator rides along the
PV matmul as a 65th lhsT column (augmented-V), which also folds the
additive attention mask in exactly (em = exp(mask) scaling of V rows).
Score psum groups pack 3 head-chunks (1536 cols) so the ACT exp — the
throughput limiter of the attention phase at 1 elem/lane/cycle — pays
its ~352-cycle per-instruction overhead a third as often.  Causal
masking is a multiplicative staircase applied after exp; the two
most-diagonal 128-key chunks per 512-query block are cropped to their
upper 256 columns (laid out so concurrently issued row-packed matmul
pairs never write the same PSUM bank).  Scores for the two heads of a
pair run concurrently via K=64 row packing.  Emission follows x-quarter
arrival so the first attention block starts ~10us in.
"""

import numpy as np

B, T, DM, H = 2, 2048, 1024, 16
HD = 64          # head dim
R = 8            # LoRA rank
NCORES = 8
GPB = 4          # head-groups (cores) per batch
HPC = 4          # heads per core
DPC = HPC * HD   # 256 output dims per core
LORA_SCALE = 2.0  # lora_alpha / r
SM_SCALE = HD ** -0.5  # 0.125

KC = DM // 128   # 8 contraction chunks
MC = DPC // 128  # 2 output-dim chunks (head pairs)
NB = 4           # t-blocks (x quarters) for q/k projections
TCH = T // 128   # 16 t-chunks (key chunks)
IBN = T // 512   # 4 query i-blocks (512 wide)
GCH = 2          # score-group capacity in 512-wide chunk units

_CACHE = {}


def _build_program():
    from contextlib import ExitStack

    import concourse.bass as bass
    import concourse.tile as tile
    from concourse import bacc, mybir

    f32 = mybir.dt.float32
    f16 = mybir.dt.float16
    EXP = mybir.ActivationFunctionType.Exp
    ts = bass.ts

    nc = bacc.Bacc(
        "TRN2",
        target_bir_lowering=False,
        debug=False,
        enable_asserts=True,
        num_devices=NCORES,
    )

    xq = nc.dram_tensor("xq", [NB, 128, KC * 512], f16, kind="ExternalInput").ap()
    wq_img = nc.dram_tensor("wq_img", [128, KC * DPC], f16, kind="ExternalInput").ap()
    wk_img = nc.dram_tensor("wk_img", [128, KC * DPC], f16, kind="ExternalInput").ap()
    wv_img = nc.dram_tensor("wv_img", [128, KC * DPC], f16, kind="ExternalInput").ap()
    a_both = nc.dram_tensor("a_both", [R, 2 * DM], f16, kind="ExternalInput").ap()
    bT_both = nc.dram_tensor("bT_both", [R, 2 * DPC], f16, kind="ExternalInput").ap()
    biasqk = nc.dram_tensor("biasqk", [128, 4], f32, kind="ExternalInput").ap()
    biasv = nc.dram_tensor("biasv", [DPC], f16, kind="ExternalInput").ap()
    amask = nc.dram_tensor("amask", [T], f32, kind="ExternalInput").ap()
    outT = nc.dram_tensor("outT", [DPC, T], f32, kind="ExternalOutput").ap()

    with tile.TileContext(nc) as tc, ExitStack() as ctx:
        const = ctx.enter_context(tc.tile_pool(name="const", bufs=1))
        xpool = ctx.enter_context(tc.tile_pool(name="x", bufs=1))
        wpool = ctx.enter_context(tc.tile_pool(name="w", bufs=1))
        qkpool = ctx.enter_context(tc.tile_pool(name="qk", bufs=1))
        vpool = ctx.enter_context(tc.tile_pool(name="v", bufs=1))
        ppool = ctx.enter_context(tc.tile_pool(name="pT", bufs=4))
        opool = ctx.enter_context(tc.tile_pool(name="osb", bufs=4))
        psum = ctx.enter_context(tc.tile_pool(name="psum", bufs=3, space="PSUM"))
        popool = ctx.enter_context(tc.tile_pool(name="po", bufs=1, space="PSUM"))

        def stair_slice(d, qo, w):
            # chunk with diagonal offset d cropped to queries [qo, qo+w):
            # valid iff j >= p + 128 d - qo; stair[p, s+j] = (j >= p) at
            # s = 384, shifted by the residue 128 d - qo (0 for every
            # crop in use, since qo = 128 d).
            start = 384 - 128 * d + qo
            return stair[:, start : start + w]

        # ---------------- loads, critical-path first ------------------
        # DMA engines drain queue descriptors roughly FIFO across
        # queues, so the small LoRA/weight transfers the first matmuls
        # depend on are issued before the 4MB of x.
        a_sb = const.tile([R, 2 * DM], f16, tag="a")
        nc.sync.dma_start(a_sb[:], a_both)
        bT_sb = const.tile([R, 2 * DPC], f16, tag="bT")
        nc.sync.dma_start(bT_sb[:], bT_both)
        wq_all = wpool.tile([128, KC * DPC], f16, tag="wq_all")
        nc.sync.dma_start(wq_all[:], wq_img)
        wk_all = wpool.tile([128, KC * DPC], f16, tag="wk_all")
        nc.sync.dma_start(wk_all[:], wk_img)

        xall = xpool.tile([128, KC * T], f16, tag="xall")
        x3d = xall[:].rearrange("p (kc c) -> p kc c", kc=KC)
        nc.scalar.dma_start(x3d[:, :, ts(0, 512)], xq[0])

        wv_all = wpool.tile([128, KC * DPC], f16, tag="wv_all")
        nc.sync.dma_start(wv_all[:], wv_img)
        bias_sb = const.tile([128, 4], f32, tag="biasqk")
        nc.sync.dma_start(bias_sb[:], biasqk)
        bv_row = const.tile([1, DPC], f16, tag="bvrow")
        nc.sync.dma_start(bv_row[:], biasv.unsqueeze(0))
        em_raw = const.tile([128, TCH], f32, tag="em_raw")
        nc.sync.dma_start(em_raw[:], amask.rearrange("(c p) -> p c", p=128))

        for q in range(1, NB):
            nc.scalar.dma_start(x3d[:, :, ts(q, 512)], xq[q])

        def xs(kc, lo, n):
            return xall[:, 2048 * kc + lo : 2048 * kc + lo + n]

        ones_1xP = const.tile([1, 128], f16, tag="ones")
        nc.vector.memset(ones_1xP[:], 1.0)

        # dummy matmuls fill the PE while the first DMAs land, warming
        # the HAM clock-gate (first ~3.4us of PE activity runs at 1.2
        # instead of 2.4 GHz) so the real stream starts at full clock
        for _ in range(16):
            wps = psum.tile([128, 128], f32, tag="sc")
            nc.tensor.matmul(wps[:], ones_1xP[:], ones_1xP[:], start=True, stop=True)

        # em[p, jb] = exp(amask[128*jb + p])
        em = const.tile([128, TCH], f32, tag="em")
        nc.scalar.activation(em[:], em_raw[:], EXP)

        # Causal staircase (multiplicative, applied after exp):
        # stair[p, m] = 1.0 if m >= p + 384 else 0.0 ; shape [128, 896].
        stair = const.tile([128, 896], f16, tag="stair")
        nc.gpsimd.memset(stair[:], 1.0)
        nc.gpsimd.affine_select(
            out=stair[:],
            in_=stair[:],
            compare_op=mybir.AluOpType.is_ge,
            fill=0.0,
            base=-384,
            pattern=[[1, 896]],
            channel_multiplier=-1,
        )

        # ---------------- weights with LoRA fold for q, v ---------------
        def load_folded(w_all, a_off, bT_off, name):
            """W'.T = W.T + A.T @ (2 B.T), one [128, KC*DPC] tile."""
            wf = wpool.tile([128, KC * DPC], f16, tag=f"wf_{name}")
            for kc in range(KC):
                dps = psum.tile([128, DPC], f32, tag="sc")
                nc.tensor.matmul(
                    dps[:],
                    a_sb[:, a_off + 128 * kc : a_off + 128 * kc + 128],
                    bT_sb[:, bT_off : bT_off + DPC],
                    start=True,
                    stop=True,
                )
                nc.vector.tensor_add(
                    wf[:, ts(kc, DPC)], w_all[:, ts(kc, DPC)], dps[:]
                )
            return wf

        wq_f = load_folded(wq_all, 0, 0, "q")

        # ---------------- projections ----------------
        # qT/kT: [d, t] with d on partitions; tile mc holds head pair
        # (2mc, 2mc+1): partitions 0-63 = head 2mc, 64-127 = head 2mc+1.
        qT_sb = [qkpool.tile([128, T], f16, tag=f"qT{mc}", name=f"qT{mc}") for mc in range(MC)]
        kT_sb = [qkpool.tile([128, T], f16, tag=f"kT{mc}", name=f"kT{mc}") for mc in range(MC)]

        def project_qk(wf, dst, bias, mc, nb):
            ps = psum.tile([128, 512], f32, tag="sc")
            for kc in range(KC):
                nc.tensor.matmul(
                    ps[:],
                    wf[:, kc * DPC + mc * 128 : kc * DPC + mc * 128 + 128],
                    xs(kc, 512 * nb, 512),
                    start=(kc == 0),
                    stop=(kc == KC - 1),
                )
            nc.vector.tensor_add(
                dst[:, ts(nb, 512)],
                ps[:],
                bias.to_broadcast((128, 512)),
            )

        # v in natural [t, d] orientation, em-scaled, with the denominator
        # (em) column appended per head: [128, 4*65].
        v2_sb = [vpool.tile([128, HPC * (HD + 1)], f16, tag=f"v2{j}", name=f"v2{j}") for j in range(TCH)]

        def project_v(jbs):
            for jb in jbs:
                ps = psum.tile([128, DPC], f32, tag="sc")
                for kc in range(KC):
                    nc.tensor.matmul(
                        ps[:],
                        xs(kc, 128 * jb, 128),
                        wv_f[:, ts(kc, DPC)],
                        start=(kc == 0),
                        stop=False,
                    )
                nc.tensor.matmul(  # + ones(t) x bias_v
                    ps[:],
                    ones_1xP[:],
                    bv_row[:],
                    start=False,
                    stop=True,
                )
                v2 = v2_sb[jb]
                em_col = em[:, jb : jb + 1]
                nc.vector.tensor_mul(
                    v2[:].rearrange("p (h c) -> p h c", h=HPC)[:, :, 0:HD],
                    ps[:].rearrange("p (h c) -> p h c", h=HPC),
                    em_col.unsqueeze(1).broadcast_to((128, HPC, HD)),
                )
                nc.vector.tensor_copy(
                    v2[:, HD : HPC * (HD + 1) : HD + 1],
                    em_col.to_broadcast((128, HPC)),
                )

        # ---------------- attention for one head pair ----------------
        def attention_ib(pr, ib):
            qT, kT = qT_sb[pr], kT_sb[pr]
            nch = 4 * ib + 4  # causal key chunks per head
            # chunk stream: (hl, jb, qoff, width, d); d = diagonal
            # offset.  Diagonal chunks are cropped to exactly the
            # causally reachable query range: d=1 -> 384@128, d=2 ->
            # 256@256, d=3 -> 128@384 (d=0 stays full so the first PV
            # write covers the whole po region).
            fulls = []
            for jb in range(4 * ib + 1):
                d = jb - 4 * ib
                for hl in (0, 1):
                    fulls.append((hl, jb, 0, 512, d))
            jd = 4 * ib
            # groups: list of (chunk, col_off).  Fulls are packed GCH
            # chunks to a psum group (column offsets 0/512) so one ACT
            # exp instruction covers up to 1024 columns.  Adjacent
            # offsets always land in different psum banks, so the
            # concurrently-running row-packed (h, h') score pairs never
            # write the same bank.  Diagonal cluster: group B holds the
            # d=2 pair (h@0, h'@512, strided exp skips the holes);
            # group A packs d=1 (384@0/512) and d=3 (128@384/896)
            # hole-free.  B is emitted before A so the d=3 chunk — the
            # accumulation-group stop — is the last PV.
            groups = []
            for i in range(0, len(fulls), GCH):
                groups.append([(c, j * 512) for j, c in enumerate(fulls[i : i + GCH])])
            groups.append([
                ((0, jd + 2, 256, 256, 2), 0),
                ((1, jd + 2, 256, 256, 2), 512),
            ])
            groups.append([
                ((0, jd + 1, 128, 384, 1), 0),
                ((1, jd + 1, 128, 384, 1), 512),
                ((0, jd + 3, 384, 128, 3), 384),
                ((1, jd + 3, 384, 128, 3), 896),
            ])

            po = [
                popool.tile([65, 512], f32, tag=f"po{hl}", name=f"po{pr}_{ib}_{hl}")
                for hl in (0, 1)
            ]
            for g in groups:
                extent = 512 if len(g) == 1 else 1024
                covered = sum(c[3] for c, _ in g)
                ps = psum.tile([128, extent], f32, tag="sc")
                for (hl, jb, qo, w, d), off in g:
                    nc.tensor.matmul(
                        ps[:, off : off + w],
                        kT[ts(hl, 64), ts(jb, 128)],
                        qT[ts(hl, 64), ib * 512 + qo : ib * 512 + qo + w],
                        start=True,
                        stop=True,
                    )
                pT = ppool.tile([128, extent], f16, tag="pT")
                if covered == extent:
                    nc.scalar.activation(pT[:], ps[:], EXP, scale=SM_SCALE)
                else:
                    # holed group (d=2 pair at 0/512): strided exp skips
                    # the stale psum columns (exp there could overflow)
                    w0 = g[0][0][3]
                    nc.scalar.activation(
                        pT[:].rearrange("p (q h) -> p q h", q=2)[:, :, 0:w0],
                        ps[:].rearrange("p (q h) -> p q h", q=2)[:, :, 0:w0],
                        EXP,
                        scale=SM_SCALE,
                    )
                # causal staircase on partial chunks; merge the
                # (h, h') twin segments (always 512 apart) into one 3-D op
                i = 0
                while i < len(g):
                    (hl, jb, qo, w, d), off_i = g[i]
                    if d < 0:
                        i += 1
                        continue
                    msk = stair_slice(d, qo, w)
                    twin = (
                        i + 1 < len(g)
                        and g[i + 1][0][1] == jb
                        and g[i + 1][0][3] == w
                        and g[i + 1][1] == off_i + 512
                    )
                    if twin:
                        seg = pT[:].rearrange("p (q h) -> p q h", q=2)[
                            :, :, off_i : off_i + w
                        ]
                        nc.vector.tensor_mul(
                            seg,
                            seg,
                            msk.unsqueeze(1).broadcast_to((128, 2, w)),
                        )
                        i += 2
                    else:
                        nc.vector.tensor_mul(
                            pT[:, off_i : off_i + w],
                            pT[:, off_i : off_i + w],
                            msk,
                        )
                        i += 1
                # PV: outT[d, i] accumulation per head; denominator
                # column (em) rides along as lhsT column 64.
                for (hl, jb, qo, w, d), off in g:
                    nc.tensor.matmul(
                        po[hl][:, qo : qo + w],
                        v2_sb[jb][:, (2 * pr + hl) * (HD + 1) : (2 * pr + hl + 1) * (HD + 1)],
                        pT[:, off : off + w],
                        start=(jb == 0),
                        stop=(jb == nch - 1),
                    )
            # normalize: out[:64] / denom (row 64), per column — both
            # heads batched through one reshape/reciprocal/broadcast
            # chain to halve the serial tail latency.
            sbp = opool.tile([65, 1024], f32, tag="sbp")
            for hl in (0, 1):
                nc.vector.tensor_copy(sbp[:, ts(hl, 512)], po[hl][:])
            # reshape the denominator row onto 128 partitions so the
            # (slow, free-size-bound) DVE reciprocal runs on [128, 8]
            dn = opool.tile([128, 8], f32, tag="dn")
            nc.scalar.dma_start(
                dn[:], sbp[64:65, :].rearrange("o (p c) -> o p c", p=128)
            )
            dnr = opool.tile([128, 8], f32, tag="dnr")
            nc.vector.reciprocal(dnr[:], dn[:])
            rc = opool.tile([1, 1024], f32, tag="rc")
            nc.sync.dma_start(
                rc[:].rearrange("o (p c) -> o p c", p=128), dnr[:]
            )
            # broadcast/multiply/store per head so the first head's
            # output DMA overlaps the second head's broadcast
            rb = opool.tile([64, 1024], f32, tag="rb")
            oT = opool.tile([64, 1024], f32, tag="oT")
            for hl in (0, 1):
                nc.gpsimd.partition_broadcast(rb[:, ts(hl, 512)], rc[:, ts(hl, 512)])
                nc.vector.tensor_mul(
                    oT[:, ts(hl, 512)], sbp[0:64, ts(hl, 512)], rb[:, ts(hl, 512)]
                )
                nc.sync.dma_start(
                    outT[(2 * pr + hl) * HD : (2 * pr + hl + 1) * HD, ts(ib, 512)],
                    oT[:, ts(hl, 512)],
                )

        # emission order follows x-quarter arrival: per quarter, pair-0
        # projections + attention, then pair-1 likewise — projections
        # act as PE filler while ACT exps the other pair's scores.  The
        # wv fold is emitted after the first q/k projections so the PE
        # queue doesn't head-of-line block on the wv DMA.
        # Per quarter: ALL projections first (their DVE bias-adds and v2
        # ops precede the attention masks in the in-order DVE queue, so
        # weight production is never head-of-line blocked behind masks),
        # then the two attention calls.
        project_qk(wq_f, qT_sb[0], bias_sb[:, 0:1], 0, 0)
        project_qk(wk_all, kT_sb[0], bias_sb[:, 2:3], 0, 0)
        wv_f = load_folded(wv_all, DM, DPC, "v")
        for nb in range(NB):
            if nb > 0:
                project_qk(wq_f, qT_sb[0], bias_sb[:, 0:1], 0, nb)
                project_qk(wk_all, kT_sb[0], bias_sb[:, 2:3], 0, nb)
            project_v(range(4 * nb, 4 * nb + 4))
            project_qk(wq_f, qT_sb[1], bias_sb[:, 1:2], 1, nb)
            project_qk(wk_all, kT_sb[1], bias_sb[:, 3:4], 1, nb)
            attention_ib(0, nb)
            attention_ib(1, nb)

    nc.compile()
    return nc


def _shard_inputs(inputs):
    """Full inputs -> per-core input maps (host-side layout + fp16 cast)."""
    f16 = np.float16
    hs = np.asarray(inputs["hidden_states"], dtype=np.float32)
    am = np.asarray(inputs["attention_mask"], dtype=np.float32)
    Wq = np.asarray(inputs["Wq"], dtype=np.float32)
    Wk = np.asarray(inputs["Wk"], dtype=np.float32)
    Wv = np.asarray(inputs["Wv"], dtype=np.float32)
    bq = np.asarray(inputs["bq"], dtype=np.float32)
    bk = np.asarray(inputs["bk"], dtype=np.float32)
    bv = np.asarray(inputs["bv"], dtype=np.float32)
    Aq = np.asarray(inputs["Aq"], dtype=np.float32)
    Bq = np.asarray(inputs["Bq"], dtype=np.float32)
    Av = np.asarray(inputs["Av"], dtype=np.float32)
    Bv = np.asarray(inputs["Bv"], dtype=np.float32)

    c = np.ascontiguousarray

    def wimg(W, sl):
        # sbuf image: wimg[p, 256*kc + j] = W[sl].T[128*kc + p, j]
        return c(W[sl].T.astype(f16).reshape(KC, 128, DPC).transpose(1, 0, 2).reshape(128, KC * DPC))

    # x quarter images: Xq[q, p, 512*kc + cc] = hs[b].T[128*kc + p, 512*q + cc]
    xqs = [
        c(hs[b].T.astype(f16).reshape(KC, 128, NB, 512).transpose(2, 1, 0, 3).reshape(NB, 128, KC * 512))
        for b in range(B)
    ]
    a_both = c(np.concatenate([Aq, Av], axis=1).astype(f16))
    in_maps = []
    for core in range(NCORES):
        b, g = core // GPB, core % GPB
        sl = slice(g * DPC, (g + 1) * DPC)
        bqk = np.stack(
            [bq[sl][:128], bq[sl][128:], bk[sl][:128], bk[sl][128:]], axis=1
        )
        bT = np.concatenate(
            [LORA_SCALE * Bq[sl].T, LORA_SCALE * Bv[sl].T], axis=1
        ).astype(f16)
        in_maps.append(
            {
                "xq": xqs[b],
                "wq_img": wimg(Wq, sl),
                "wk_img": wimg(Wk, sl),
                "wv_img": wimg(Wv, sl),
                "a_both": a_both,
                "bT_both": c(bT),
                "biasqk": c(bqk),
                "biasv": c(bv[sl].astype(f16)),
                "amask": c(am[b, 0, 0, :]),
            }
        )
    return in_maps


def _run(inputs, trace=False):
    from concourse.bass_utils import run_bass_kernel_spmd

    if "nc" not in _CACHE:
        _CACHE["nc"] = _build_program()
    nc = _CACHE["nc"]
    in_maps = _shard_inputs(inputs)
    res = run_bass_kernel_spmd(nc, in_maps, list(range(NCORES)), trace=trace)
    out = np.empty((B, T, DM), dtype=np.float32)
    for core in range(NCORES):
        b, g = core // GPB, core % GPB
        out[b, :, g * DPC : (g + 1) * DPC] = res.results[core]["outT"].T
    return out, res


def kernel(**inputs) -> np.ndarray:
    out, _ = _run(inputs, trace=False)
    return out


# revision 34
# speedup vs baseline: 1.0415x; 1.0112x over previous
"""Causal self-attention with LoRA (q,v) — Trainium2 Bass kernel, 8 cores.

Sharding: data-parallel over batch (B=2), tensor-parallel over heads
(16 heads -> 4 per core).  Core c handles batch c//4, heads 4*(c%4)..+4.
Each core computes its 256-dim q/k/v projection slice from the full
hidden states and its heads' full 2048x2048 causal attention locally.
No collectives; host does the (layout-only) scatter/gather + fp16 casts
+ sbuf-image tiling so every bulk load is one contiguous HWDGE DMA.

The whole datapath runs in fp16 with fp32 PSUM accumulation (measured
rel err vs the fp32 reference ~5e-4).  fp16 matters a lot on TRN2: f32r
matmuls cannot use standalone LDWEIGHTS (walrus bug) so every f32r MM
pays an inline ~214ns weight load, and FP32_HIGH disables fast weight
load entirely; fp16 streams at 1 col/cycle with LDW hidden by the PE
reorder window.

Attention is computed in transposed orientation (scores sT[j, i]) so no
on-chip transposes are needed; the softmax denominator rides along the
PV matmul as a 65th lhsT column (augmented-V), which also folds the
additive attention mask in exactly (em = exp(mask) scaling of V rows).
Score psum groups pack 3 head-chunks (1536 cols) so the ACT exp — the
throughput limiter of the attention phase at 1 elem/lane/cycle — pays
its ~352-cycle per-instruction overhead a third as often.  Causal
masking is a multiplicative staircase applied after exp; the two
most-diagonal 128-key chunks per 512-query block are cropped to their
upper 256 columns (laid out so concurrently issued row-packed matmul
pairs never write the same PSUM bank).  Scores for the two heads of a
pair run concurrently via K=64 row packing.  Emission follows x-quarter
arrival so the first attention block starts ~10us in.
"""

import numpy as np

B, T, DM, H = 2, 2048, 1024, 16
HD = 64          # head dim
R = 8            # LoRA rank
NCORES = 8
GPB = 4          # head-groups (cores) per batch
HPC = 4          # heads per core
DPC = HPC * HD   # 256 output dims per core
LORA_SCALE = 2.0  # lora_alpha / r
SM_SCALE = HD ** -0.5  # 0.125

KC = DM // 128   # 8 contraction chunks
MC = DPC // 128  # 2 output-dim chunks (head pairs)
NB = 4           # t-blocks (x quarters) for q/k projections
TCH = T // 128   # 16 t-chunks (key chunks)
IBN = T // 512   # 4 query i-blocks (512 wide)
GCH = 2          # score-group capacity in 512-wide chunk units

_CACHE = {}


def _build_program():
    from contextlib import ExitStack

    import concourse.bass as bass
    import concourse.tile as tile
    from concourse import bacc, mybir

    f32 = mybir.dt.float32
    f16 = mybir.dt.float16
    EXP = mybir.ActivationFunctionType.Exp
    ts = bass.ts

    nc = bacc.Bacc(
        "TRN2",
        target_bir_lowering=False,
        debug=False,
        enable_asserts=True,
        num_devices=NCORES,
    )

    xq = nc.dram_tensor("xq", [NB, 128, KC * 512], f16, kind="ExternalInput").ap()
    wq_img = nc.dram_tensor("wq_img", [128, KC * DPC], f16, kind="ExternalInput").ap()
    wk_img = nc.dram_tensor("wk_img", [128, KC * DPC], f16, kind="ExternalInput").ap()
    wv_img = nc.dram_tensor("wv_img", [128, KC * DPC], f16, kind="ExternalInput").ap()
    a_both = nc.dram_tensor("a_both", [R, 2 * DM], f16, kind="ExternalInput").ap()
    bT_both = nc.dram_tensor("bT_both", [R, 2 * DPC], f16, kind="ExternalInput").ap()
    biasqk = nc.dram_tensor("biasqk", [128, 4], f32, kind="ExternalInput").ap()
    biasv = nc.dram_tensor("biasv", [DPC], f16, kind="ExternalInput").ap()
    amask = nc.dram_tensor("amask", [T], f32, kind="ExternalInput").ap()
    outT = nc.dram_tensor("outT", [DPC, T], f32, kind="ExternalOutput").ap()

    with tile.TileContext(nc) as tc, ExitStack() as ctx:
        const = ctx.enter_context(tc.tile_pool(name="const", bufs=1))
        xpool = ctx.enter_context(tc.tile_pool(name="x", bufs=1))
        wpool = ctx.enter_context(tc.tile_pool(name="w", bufs=1))
        qkpool = ctx.enter_context(tc.tile_pool(name="qk", bufs=1))
        vpool = ctx.enter_context(tc.tile_pool(name="v", bufs=1))
        ppool = ctx.enter_context(tc.tile_pool(name="pT", bufs=4))
        opool = ctx.enter_context(tc.tile_pool(name="osb", bufs=4))
        psum = ctx.enter_context(tc.tile_pool(name="psum", bufs=3, space="PSUM"))
        popool = ctx.enter_context(tc.tile_pool(name="po", bufs=1, space="PSUM"))

        def stair_slice(d, qo, w):
            # chunk with diagonal offset d cropped to queries [qo, qo+w):
            # valid iff j >= p + 128 d - qo; stair[p, s+j] = (j >= p) at
            # s = 384, shifted by the residue 128 d - qo (0 for every
            # crop in use, since qo = 128 d).
            start = 384 - 128 * d + qo
            return stair[:, start : start + w]

        # ---------------- loads, critical-path first ------------------
        # DMA engines drain queue descriptors roughly FIFO across
        # queues, so the small LoRA/weight transfers the first matmuls
        # depend on are issued before the 4MB of x.
        a_sb = const.tile([R, 2 * DM], f16, tag="a")
        nc.sync.dma_start(a_sb[:], a_both)
        bT_sb = const.tile([R, 2 * DPC], f16, tag="bT")
        nc.sync.dma_start(bT_sb[:], bT_both)
        wq_all = wpool.tile([128, KC * DPC], f16, tag="wq_all")
        nc.sync.dma_start(wq_all[:], wq_img)
        wk_all = wpool.tile([128, KC * DPC], f16, tag="wk_all")
        nc.sync.dma_start(wk_all[:], wk_img)

        xall = xpool.tile([128, KC * T], f16, tag="xall")
        x3d = xall[:].rearrange("p (kc c) -> p kc c", kc=KC)
        # first quarter split in two so the first projection matmuls
        # (kc 0-3) can start after 512KB instead of 1MB
        nc.scalar.dma_start(x3d[:, 0:4, ts(0, 512)], xq[0][:, 0 : 4 * 512])
        nc.scalar.dma_start(x3d[:, 4:8, ts(0, 512)], xq[0][:, 4 * 512 : 8 * 512])

        wv_all = wpool.tile([128, KC * DPC], f16, tag="wv_all")
        nc.sync.dma_start(wv_all[:], wv_img)
        bias_sb = const.tile([128, 4], f32, tag="biasqk")
        nc.sync.dma_start(bias_sb[:], biasqk)
        bv_row = const.tile([1, DPC], f16, tag="bvrow")
        nc.sync.dma_start(bv_row[:], biasv.unsqueeze(0))
        em_raw = const.tile([128, TCH], f32, tag="em_raw")
        nc.sync.dma_start(em_raw[:], amask.rearrange("(c p) -> p c", p=128))

        for q in range(1, NB):
            nc.scalar.dma_start(x3d[:, :, ts(q, 512)], xq[q])

        def xs(kc, lo, n):
            return xall[:, 2048 * kc + lo : 2048 * kc + lo + n]

        ones_1xP = const.tile([1, 128], f16, tag="ones")
        nc.vector.memset(ones_1xP[:], 1.0)

        # dummy matmuls fill the PE while the first DMAs land, warming
        # the HAM clock-gate (first ~3.4us of PE activity runs at 1.2
        # instead of 2.4 GHz) so the real stream starts at full clock
        for _ in range(16):
            wps = psum.tile([128, 128], f32, tag="sc")
            nc.tensor.matmul(wps[:], ones_1xP[:], ones_1xP[:], start=True, stop=True)

        # em[p, jb] = exp(amask[128*jb + p])
        em = const.tile([128, TCH], f32, tag="em")
        nc.scalar.activation(em[:], em_raw[:], EXP)

        # Causal staircase (multiplicative, applied after exp):
        # stair[p, m] = 1.0 if m >= p + 384 else 0.0 ; shape [128, 896].
        stair = const.tile([128, 896], f16, tag="stair")
        nc.gpsimd.memset(stair[:], 1.0)
        nc.gpsimd.affine_select(
            out=stair[:],
            in_=stair[:],
            compare_op=mybir.AluOpType.is_ge,
            fill=0.0,
            base=-384,
            pattern=[[1, 896]],
            channel_multiplier=-1,
        )

        # ---------------- weights with LoRA fold for q, v ---------------
        def load_folded(w_all, a_off, bT_off, name):
            """W'.T = W.T + A.T @ (2 B.T), one [128, KC*DPC] tile."""
            wf = wpool.tile([128, KC * DPC], f16, tag=f"wf_{name}")
            for kc in range(KC):
                dps = psum.tile([128, DPC], f32, tag="sc")
                nc.tensor.matmul(
                    dps[:],
                    a_sb[:, a_off + 128 * kc : a_off + 128 * kc + 128],
                    bT_sb[:, bT_off : bT_off + DPC],
                    start=True,
                    stop=True,
                )
                nc.vector.tensor_add(
                    wf[:, ts(kc, DPC)], w_all[:, ts(kc, DPC)], dps[:]
                )
            return wf

        wq_f = load_folded(wq_all, 0, 0, "q")

        # ---------------- projections ----------------
        # qT/kT: [d, t] with d on partitions; tile mc holds head pair
        # (2mc, 2mc+1): partitions 0-63 = head 2mc, 64-127 = head 2mc+1.
        qT_sb = [qkpool.tile([128, T], f16, tag=f"qT{mc}", name=f"qT{mc}") for mc in range(MC)]
        kT_sb = [qkpool.tile([128, T], f16, tag=f"kT{mc}", name=f"kT{mc}") for mc in range(MC)]

        def project_qk(wf, dst, bias, mc, nb):
            ps = psum.tile([128, 512], f32, tag="sc")
            for kc in range(KC):
                nc.tensor.matmul(
                    ps[:],
                    wf[:, kc * DPC + mc * 128 : kc * DPC + mc * 128 + 128],
                    xs(kc, 512 * nb, 512),
                    start=(kc == 0),
                    stop=(kc == KC - 1),
                )
            nc.vector.tensor_add(
                dst[:, ts(nb, 512)],
                ps[:],
                bias.to_broadcast((128, 512)),
            )

        # v in natural [t, d] orientation, em-scaled, with the denominator
        # (em) column appended per head: [128, 4*65].
        v2_sb = [vpool.tile([128, HPC * (HD + 1)], f16, tag=f"v2{j}", name=f"v2{j}") for j in range(TCH)]

        def project_v(jbs):
            for jb in jbs:
                ps = psum.tile([128, DPC], f32, tag="sc")
                for kc in range(KC):
                    nc.tensor.matmul(
                        ps[:],
                        xs(kc, 128 * jb, 128),
                        wv_f[:, ts(kc, DPC)],
                        start=(kc == 0),
                        stop=False,
                    )
                nc.tensor.matmul(  # + ones(t) x bias_v
                    ps[:],
                    ones_1xP[:],
                    bv_row[:],
                    start=False,
                    stop=True,
                )
                v2 = v2_sb[jb]
                em_col = em[:, jb : jb + 1]
                nc.vector.tensor_mul(
                    v2[:].rearrange("p (h c) -> p h c", h=HPC)[:, :, 0:HD],
                    ps[:].rearrange("p (h c) -> p h c", h=HPC),
                    em_col.unsqueeze(1).broadcast_to((128, HPC, HD)),
                )
                nc.vector.tensor_copy(
                    v2[:, HD : HPC * (HD + 1) : HD + 1],
                    em_col.to_broadcast((128, HPC)),
                )

        # ---------------- attention for one head pair ----------------
        def attention_ib(pr, ib):
            qT, kT = qT_sb[pr], kT_sb[pr]
            nch = 4 * ib + 4  # causal key chunks per head
            # chunk stream: (hl, jb, qoff, width, d); d = diagonal
            # offset.  Diagonal chunks are cropped to exactly the
            # causally reachable query range: d=1 -> 384@128, d=2 ->
            # 256@256, d=3 -> 128@384 (d=0 stays full so the first PV
            # write covers the whole po region).
            fulls = []
            for jb in range(4 * ib + 1):
                d = jb - 4 * ib
                for hl in (0, 1):
                    fulls.append((hl, jb, 0, 512, d))
            jd = 4 * ib
            # groups: list of (chunk, col_off).  Fulls are packed GCH
            # chunks to a psum group (column offsets 0/512) so one ACT
            # exp instruction covers up to 1024 columns.  Adjacent
            # offsets always land in different psum banks, so the
            # concurrently-running row-packed (h, h') score pairs never
            # write the same bank.  Diagonal cluster: group B holds the
            # d=2 pair (h@0, h'@512, strided exp skips the holes);
            # group A packs d=1 (384@0/512) and d=3 (128@384/896)
            # hole-free.  B is emitted before A so the d=3 chunk — the
            # accumulation-group stop — is the last PV.
            groups = []
            for i in range(0, len(fulls), GCH):
                groups.append([(c, j * 512) for j, c in enumerate(fulls[i : i + GCH])])
            groups.append([
                ((0, jd + 2, 256, 256, 2), 0),
                ((1, jd + 2, 256, 256, 2), 512),
            ])
            groups.append([
                ((0, jd + 1, 128, 384, 1), 0),
                ((1, jd + 1, 128, 384, 1), 512),
                ((0, jd + 3, 384, 128, 3), 384),
                ((1, jd + 3, 384, 128, 3), 896),
            ])

            po = [
                popool.tile([65, 512], f32, tag=f"po{hl}", name=f"po{pr}_{ib}_{hl}")
                for hl in (0, 1)
            ]
            for g in groups:
                extent = 512 if len(g) == 1 else 1024
                covered = sum(c[3] for c, _ in g)
                ps = psum.tile([128, extent], f32, tag="sc")
                for (hl, jb, qo, w, d), off in g:
                    nc.tensor.matmul(
                        ps[:, off : off + w],
                        kT[ts(hl, 64), ts(jb, 128)],
                        qT[ts(hl, 64), ib * 512 + qo : ib * 512 + qo + w],
                        start=True,
                        stop=True,
                    )
                pT = ppool.tile([128, extent], f16, tag="pT")
                if covered == extent:
                    nc.scalar.activation(pT[:], ps[:], EXP, scale=SM_SCALE)
                else:
                    # holed group (d=2 pair at 0/512): strided exp skips
                    # the stale psum columns (exp there could overflow)
                    w0 = g[0][0][3]
                    nc.scalar.activation(
                        pT[:].rearrange("p (q h) -> p q h", q=2)[:, :, 0:w0],
                        ps[:].rearrange("p (q h) -> p q h", q=2)[:, :, 0:w0],
                        EXP,
                        scale=SM_SCALE,
                    )
                # causal staircase on partial chunks; merge the
                # (h, h') twin segments (always 512 apart) into one 3-D op
                i = 0
                while i < len(g):
                    (hl, jb, qo, w, d), off_i = g[i]
                    if d < 0:
                        i += 1
                        continue
                    msk = stair_slice(d, qo, w)
                    twin = (
                        i + 1 < len(g)
                        and g[i + 1][0][1] == jb
                        and g[i + 1][0][3] == w
                        and g[i + 1][1] == off_i + 512
                    )
                    if twin:
                        seg = pT[:].rearrange("p (q h) -> p q h", q=2)[
                            :, :, off_i : off_i + w
                        ]
                        nc.vector.tensor_mul(
                            seg,
                            seg,
                            msk.unsqueeze(1).broadcast_to((128, 2, w)),
                        )
                        i += 2
                    else:
                        nc.vector.tensor_mul(
                            pT[:, off_i : off_i + w],
                            pT[:, off_i : off_i + w],
                            msk,
                        )
                        i += 1
                # PV: outT[d, i] accumulation per head; denominator
                # column (em) rides along as lhsT column 64.
                for (hl, jb, qo, w, d), off in g:
                    nc.tensor.matmul(
                        po[hl][:, qo : qo + w],
                        v2_sb[jb][:, (2 * pr + hl) * (HD + 1) : (2 * pr + hl + 1) * (HD + 1)],
                        pT[:, off : off + w],
                        start=(jb == 0),
                        stop=(jb == nch - 1),
                    )
            # normalize: out[:64] / denom (row 64), per column — both
            # heads batched through one reshape/reciprocal/broadcast
            # chain to halve the serial tail latency.
            sbp = opool.tile([65, 1024], f32, tag="sbp")
            for hl in (0, 1):
                nc.vector.tensor_copy(sbp[:, ts(hl, 512)], po[hl][:])
            # reshape the denominator row onto 128 partitions so the
            # (slow, free-size-bound) DVE reciprocal runs on [128, 8]
            dn = opool.tile([128, 8], f32, tag="dn")
            nc.scalar.dma_start(
                dn[:], sbp[64:65, :].rearrange("o (p c) -> o p c", p=128)
            )
            dnr = opool.tile([128, 8], f32, tag="dnr")
            nc.vector.reciprocal(dnr[:], dn[:])
            rc = opool.tile([1, 1024], f32, tag="rc")
            nc.sync.dma_start(
                rc[:].rearrange("o (p c) -> o p c", p=128), dnr[:]
            )
            # broadcast/multiply/store per head so the first head's
            # output DMA overlaps the second head's broadcast
            rb = opool.tile([64, 1024], f32, tag="rb")
            oT = opool.tile([64, 1024], f32, tag="oT")
            for hl in (0, 1):
                nc.gpsimd.partition_broadcast(rb[:, ts(hl, 512)], rc[:, ts(hl, 512)])
                nc.vector.tensor_mul(
                    oT[:, ts(hl, 512)], sbp[0:64, ts(hl, 512)], rb[:, ts(hl, 512)]
                )
                nc.sync.dma_start(
                    outT[(2 * pr + hl) * HD : (2 * pr + hl + 1) * HD, ts(ib, 512)],
                    oT[:, ts(hl, 512)],
                )

        # emission order follows x-quarter arrival: per quarter, pair-0
        # projections + attention, then pair-1 likewise — projections
        # act as PE filler while ACT exps the other pair's scores.  The
        # wv fold is emitted after the first q/k projections so the PE
        # queue doesn't head-of-line block on the wv DMA.
        # Per quarter: ALL projections first (their DVE bias-adds and v2
        # ops precede the attention masks in the in-order DVE queue, so
        # weight production is never head-of-line blocked behind masks),
        # then the two attention calls.
        project_qk(wq_f, qT_sb[0], bias_sb[:, 0:1], 0, 0)
        project_qk(wk_all, kT_sb[0], bias_sb[:, 2:3], 0, 0)
        wv_f = load_folded(wv_all, DM, DPC, "v")
        for nb in range(NB):
            if nb > 0:
                project_qk(wq_f, qT_sb[0], bias_sb[:, 0:1], 0, nb)
                project_qk(wk_all, kT_sb[0], bias_sb[:, 2:3], 0, nb)
            project_v(range(4 * nb, 4 * nb + 4))
            project_qk(wq_f, qT_sb[1], bias_sb[:, 1:2], 1, nb)
            project_qk(wk_all, kT_sb[1], bias_sb[:, 3:4], 1, nb)
            attention_ib(0, nb)
            attention_ib(1, nb)

    nc.compile()
    return nc


def _shard_inputs(inputs):
    """Full inputs -> per-core input maps (host-side layout + fp16 cast)."""
    f16 = np.float16
    hs = np.asarray(inputs["hidden_states"], dtype=np.float32)
    am = np.asarray(inputs["attention_mask"], dtype=np.float32)
    Wq = np.asarray(inputs["Wq"], dtype=np.float32)
    Wk = np.asarray(inputs["Wk"], dtype=np.float32)
    Wv = np.asarray(inputs["Wv"], dtype=np.float32)
    bq = np.asarray(inputs["bq"], dtype=np.float32)
    bk = np.asarray(inputs["bk"], dtype=np.float32)
    bv = np.asarray(inputs["bv"], dtype=np.float32)
    Aq = np.asarray(inputs["Aq"], dtype=np.float32)
    Bq = np.asarray(inputs["Bq"], dtype=np.float32)
    Av = np.asarray(inputs["Av"], dtype=np.float32)
    Bv = np.asarray(inputs["Bv"], dtype=np.float32)

    c = np.ascontiguousarray

    def wimg(W, sl):
        # sbuf image: wimg[p, 256*kc + j] = W[sl].T[128*kc + p, j]
        return c(W[sl].T.astype(f16).reshape(KC, 128, DPC).transpose(1, 0, 2).reshape(128, KC * DPC))

    # x quarter images: Xq[q, p, 512*kc + cc] = hs[b].T[128*kc + p, 512*q + cc]
    xqs = [
        c(hs[b].T.astype(f16).reshape(KC, 128, NB, 512).transpose(2, 1, 0, 3).reshape(NB, 128, KC * 512))
        for b in range(B)
    ]
    a_both = c(np.concatenate([Aq, Av], axis=1).astype(f16))
    in_maps = []
    for core in range(NCORES):
        b, g = core // GPB, core % GPB
        sl = slice(g * DPC, (g + 1) * DPC)
        bqk = np.stack(
            [bq[sl][:128], bq[sl][128:], bk[sl][:128], bk[sl][128:]], axis=1
        )
        bT = np.concatenate(
            [LORA_SCALE * Bq[sl].T, LORA_SCALE * Bv[sl].T], axis=1
        ).astype(f16)
        in_maps.append(
            {
                "xq": xqs[b],
                "wq_img": wimg(Wq, sl),
                "wk_img": wimg(Wk, sl),
                "wv_img": wimg(Wv, sl),
                "a_both": a_both,
                "bT_both": c(bT),
                "biasqk": c(bqk),
                "biasv": c(bv[sl].astype(f16)),
                "amask": c(am[b, 0, 0, :]),
            }
        )
    return in_maps


def _run(inputs, trace=False):
    from concourse.bass_utils import run_bass_kernel_spmd

    if "nc" not in _CACHE:
        _CACHE["nc"] = _build_program()
    nc = _CACHE["nc"]
    in_maps = _shard_inputs(inputs)
    res = run_bass_kernel_spmd(nc, in_maps, list(range(NCORES)), trace=trace)
    out = np.empty((B, T, DM), dtype=np.float32)
    for core in range(NCORES):
        b, g = core // GPB, core % GPB
        out[b, :, g * DPC : (g + 1) * DPC] = res.results[core]["outT"].T
    return out, res


def kernel(**inputs) -> np.ndarray:
    out, _ = _run(inputs, trace=False)
    return out


# revision 36
# speedup vs baseline: 1.0519x; 1.0100x over previous
"""Causal self-attention with LoRA (q,v) — Trainium2 Bass kernel, 8 cores.

Sharding: data-parallel over batch (B=2), tensor-parallel over heads
(16 heads -> 4 per core).  Core c handles batch c//4, heads 4*(c%4)..+4.
Each core computes its 256-dim q/k/v projection slice from the full
hidden states and its heads' full 2048x2048 causal attention locally.
No collectives; host does the (layout-only) scatter/gather + fp16 casts
+ sbuf-image tiling so every bulk load is one contiguous HWDGE DMA.

The whole datapath runs in fp16 with fp32 PSUM accumulation (measured
rel err vs the fp32 reference ~5e-4).  fp16 matters a lot on TRN2: f32r
matmuls cannot use standalone LDWEIGHTS (walrus bug) so every f32r MM
pays an inline ~214ns weight load, and FP32_HIGH disables fast weight
load entirely; fp16 streams at 1 col/cycle with LDW hidden by the PE
reorder window.

Attention is computed in transposed orientation (scores sT[j, i]) so no
on-chip transposes are needed; the softmax denominator rides along the
PV matmul as a 65th lhsT column (augmented-V), which also folds the
additive attention mask in exactly (em = exp(mask) scaling of V rows).
Score psum groups pack 3 head-chunks (1536 cols) so the ACT exp — the
throughput limiter of the attention phase at 1 elem/lane/cycle — pays
its ~352-cycle per-instruction overhead a third as often.  Causal
masking is a multiplicative staircase applied after exp; the two
most-diagonal 128-key chunks per 512-query block are cropped to their
upper 256 columns (laid out so concurrently issued row-packed matmul
pairs never write the same PSUM bank).  Scores for the two heads of a
pair run concurrently via K=64 row packing.  Emission follows x-quarter
arrival so the first attention block starts ~10us in.
"""

import numpy as np

B, T, DM, H = 2, 2048, 1024, 16
HD = 64          # head dim
R = 8            # LoRA rank
NCORES = 8
GPB = 4          # head-groups (cores) per batch
HPC = 4          # heads per core
DPC = HPC * HD   # 256 output dims per core
LORA_SCALE = 2.0  # lora_alpha / r
SM_SCALE = HD ** -0.5  # 0.125

KC = DM // 128   # 8 contraction chunks
MC = DPC // 128  # 2 output-dim chunks (head pairs)
NB = 4           # t-blocks (x quarters) for q/k projections
TCH = T // 128   # 16 t-chunks (key chunks)
IBN = T // 512   # 4 query i-blocks (512 wide)
GCH = 2          # score-group capacity in 512-wide chunk units

_CACHE = {}


def _build_program():
    from contextlib import ExitStack

    import concourse.bass as bass
    import concourse.tile as tile
    from concourse import bacc, mybir

    f32 = mybir.dt.float32
    f16 = mybir.dt.float16
    EXP = mybir.ActivationFunctionType.Exp
    ts = bass.ts

    nc = bacc.Bacc(
        "TRN2",
        target_bir_lowering=False,
        debug=False,
        enable_asserts=True,
        num_devices=NCORES,
    )

    xq = nc.dram_tensor("xq", [NB, 128, KC * 512], f16, kind="ExternalInput").ap()
    wq_img = nc.dram_tensor("wq_img", [128, KC * DPC], f16, kind="ExternalInput").ap()
    wk_img = nc.dram_tensor("wk_img", [128, KC * DPC], f16, kind="ExternalInput").ap()
    wv_img = nc.dram_tensor("wv_img", [128, KC * DPC], f16, kind="ExternalInput").ap()
    a_both = nc.dram_tensor("a_both", [R, 2 * DM], f16, kind="ExternalInput").ap()
    bT_both = nc.dram_tensor("bT_both", [R, 2 * DPC], f16, kind="ExternalInput").ap()
    biasqk = nc.dram_tensor("biasqk", [128, 4], f32, kind="ExternalInput").ap()
    biasv = nc.dram_tensor("biasv", [DPC], f16, kind="ExternalInput").ap()
    amask = nc.dram_tensor("amask", [T], f32, kind="ExternalInput").ap()
    outT = nc.dram_tensor("outT", [DPC, T], f32, kind="ExternalOutput").ap()

    with tile.TileContext(nc) as tc, ExitStack() as ctx:
        const = ctx.enter_context(tc.tile_pool(name="const", bufs=1))
        xpool = ctx.enter_context(tc.tile_pool(name="x", bufs=1))
        wpool = ctx.enter_context(tc.tile_pool(name="w", bufs=1))
        qkpool = ctx.enter_context(tc.tile_pool(name="qk", bufs=1))
        vpool = ctx.enter_context(tc.tile_pool(name="v", bufs=1))
        ppool = ctx.enter_context(tc.tile_pool(name="pT", bufs=4))
        opool = ctx.enter_context(tc.tile_pool(name="osb", bufs=4))
        psum = ctx.enter_context(tc.tile_pool(name="psum", bufs=3, space="PSUM"))
        popool = ctx.enter_context(tc.tile_pool(name="po", bufs=1, space="PSUM"))

        def stair_slice(d, qo, w):
            # chunk with diagonal offset d cropped to queries [qo, qo+w):
            # valid iff j >= p + 128 d - qo; stair[p, s+j] = (j >= p) at
            # s = 384, shifted by the residue 128 d - qo (0 for every
            # crop in use, since qo = 128 d).
            start = 384 - 128 * d + qo
            return stair[:, start : start + w]

        # ---------------- loads, critical-path first ------------------
        # DMA engines drain queue descriptors roughly FIFO across
        # queues, so the small LoRA/weight transfers the first matmuls
        # depend on are issued before the 4MB of x.
        a_sb = const.tile([R, 2 * DM], f16, tag="a")
        nc.sync.dma_start(a_sb[:], a_both)
        bT_sb = const.tile([R, 2 * DPC], f16, tag="bT")
        nc.sync.dma_start(bT_sb[:], bT_both)
        wq_all = wpool.tile([128, KC * DPC], f16, tag="wq_all")
        nc.sync.dma_start(wq_all[:], wq_img)
        wk_all = wpool.tile([128, KC * DPC], f16, tag="wk_all")
        nc.sync.dma_start(wk_all[:], wk_img)

        xall = xpool.tile([128, KC * T], f16, tag="xall")
        x3d = xall[:].rearrange("p (kc c) -> p kc c", kc=KC)
        # first quarter split in two so the first projection matmuls
        # (kc 0-3) can start after 512KB instead of 1MB
        nc.scalar.dma_start(x3d[:, 0:4, ts(0, 512)], xq[0][:, 0 : 4 * 512])
        nc.scalar.dma_start(x3d[:, 4:8, ts(0, 512)], xq[0][:, 4 * 512 : 8 * 512])

        wv_all = wpool.tile([128, KC * DPC], f16, tag="wv_all")
        nc.sync.dma_start(wv_all[:], wv_img)
        bias_sb = const.tile([128, 4], f32, tag="biasqk")
        nc.sync.dma_start(bias_sb[:], biasqk)
        bv_row = const.tile([1, DPC], f16, tag="bvrow")
        nc.sync.dma_start(bv_row[:], biasv.unsqueeze(0))
        em_raw = const.tile([128, TCH], f32, tag="em_raw")
        nc.sync.dma_start(em_raw[:], amask.rearrange("(c p) -> p c", p=128))

        for q in range(1, NB):
            nc.scalar.dma_start(x3d[:, :, ts(q, 512)], xq[q])

        def xs(kc, lo, n):
            return xall[:, 2048 * kc + lo : 2048 * kc + lo + n]

        ones_1xP = const.tile([1, 128], f16, tag="ones")
        nc.vector.memset(ones_1xP[:], 1.0)

        # dummy matmuls fill the PE while the first DMAs land, warming
        # the HAM clock-gate (first ~3.4us of PE activity runs at 1.2
        # instead of 2.4 GHz) so the real stream starts at full clock
        for _ in range(16):
            wps = psum.tile([128, 128], f32, tag="sc")
            nc.tensor.matmul(wps[:], ones_1xP[:], ones_1xP[:], start=True, stop=True)

        # em[p, jb] = exp(amask[128*jb + p])
        em = const.tile([128, TCH], f32, tag="em")
        nc.scalar.activation(em[:], em_raw[:], EXP)

        # Causal staircase (multiplicative, applied after exp):
        # stair[p, m] = 1.0 if m >= p + 384 else 0.0 ; shape [128, 896].
        stair = const.tile([128, 896], f16, tag="stair")
        nc.gpsimd.memset(stair[:], 1.0)
        nc.gpsimd.affine_select(
            out=stair[:],
            in_=stair[:],
            compare_op=mybir.AluOpType.is_ge,
            fill=0.0,
            base=-384,
            pattern=[[1, 896]],
            channel_multiplier=-1,
        )

        # ---------------- weights with LoRA fold for q, v ---------------
        def load_folded(w_all, a_off, bT_off, name):
            """W'.T = W.T + A.T @ (2 B.T), one [128, KC*DPC] tile."""
            wf = wpool.tile([128, KC * DPC], f16, tag=f"wf_{name}")
            for kc in range(KC):
                dps = psum.tile([128, DPC], f32, tag="sc")
                nc.tensor.matmul(
                    dps[:],
                    a_sb[:, a_off + 128 * kc : a_off + 128 * kc + 128],
                    bT_sb[:, bT_off : bT_off + DPC],
                    start=True,
                    stop=True,
                )
                nc.vector.tensor_add(
                    wf[:, ts(kc, DPC)], w_all[:, ts(kc, DPC)], dps[:]
                )
            return wf

        wq_f = load_folded(wq_all, 0, 0, "q")

        # ---------------- projections ----------------
        # qT/kT: [d, t] with d on partitions; tile mc holds head pair
        # (2mc, 2mc+1): partitions 0-63 = head 2mc, 64-127 = head 2mc+1.
        qT_sb = [qkpool.tile([128, T], f16, tag=f"qT{mc}", name=f"qT{mc}") for mc in range(MC)]
        kT_sb = [qkpool.tile([128, T], f16, tag=f"kT{mc}", name=f"kT{mc}") for mc in range(MC)]

        def project_qk(wf, dst, bias, mc, nb):
            ps = psum.tile([128, 512], f32, tag="sc")
            for kc in range(KC):
                nc.tensor.matmul(
                    ps[:],
                    wf[:, kc * DPC + mc * 128 : kc * DPC + mc * 128 + 128],
                    xs(kc, 512 * nb, 512),
                    start=(kc == 0),
                    stop=(kc == KC - 1),
                )
            nc.vector.tensor_add(
                dst[:, ts(nb, 512)],
                ps[:],
                bias.to_broadcast((128, 512)),
            )

        # v in natural [t, d] orientation, em-scaled, with the denominator
        # (em) column appended per head: [128, 4*65].
        v2_sb = [vpool.tile([128, HPC * (HD + 1)], f16, tag=f"v2{j}", name=f"v2{j}") for j in range(TCH)]

        def project_v(jbs):
            for jb in jbs:
                ps = psum.tile([128, DPC], f32, tag="sc")
                for kc in range(KC):
                    nc.tensor.matmul(
                        ps[:],
                        xs(kc, 128 * jb, 128),
                        wv_f[:, ts(kc, DPC)],
                        start=(kc == 0),
                        stop=False,
                    )
                nc.tensor.matmul(  # + ones(t) x bias_v
                    ps[:],
                    ones_1xP[:],
                    bv_row[:],
                    start=False,
                    stop=True,
                )
                v2 = v2_sb[jb]
                em_col = em[:, jb : jb + 1]
                nc.vector.tensor_mul(
                    v2[:].rearrange("p (h c) -> p h c", h=HPC)[:, :, 0:HD],
                    ps[:].rearrange("p (h c) -> p h c", h=HPC),
                    em_col.unsqueeze(1).broadcast_to((128, HPC, HD)),
                )
                nc.vector.tensor_copy(
                    v2[:, HD : HPC * (HD + 1) : HD + 1],
                    em_col.to_broadcast((128, HPC)),
                )

        def act_reciprocal(out, in_):
            # raw InstActivation: bass's activation() refuses Reciprocal
            # (spline accuracy ~1e-3 rel), fine for the final 1/denom
            # given the ~40x error margin; used only on the last call's
            # tail where it replaces two serial sbuf-reshape DMAs.
            eng = nc.scalar
            ins = [eng.lower_ap(in_)]
            for v in (0.0, 1.0, 0.0):
                ins.append(mybir.ImmediateValue(dtype=mybir.dt.float32, value=v))
            return eng.add_instruction(
                mybir.InstActivation(
                    name=nc.get_next_instruction_name(),
                    func=mybir.ActivationFunctionType.Reciprocal,
                    ins=ins,
                    outs=[eng.lower_ap(out)],
                )
            )

        # ---------------- attention for one head pair ----------------
        def attention_ib(pr, ib):
            qT, kT = qT_sb[pr], kT_sb[pr]
            nch = 4 * ib + 4  # causal key chunks per head
            # chunk stream: (hl, jb, qoff, width, d); d = diagonal
            # offset.  Diagonal chunks are cropped to exactly the
            # causally reachable query range: d=1 -> 384@128, d=2 ->
            # 256@256, d=3 -> 128@384 (d=0 stays full so the first PV
            # write covers the whole po region).
            fulls = []
            for jb in range(4 * ib + 1):
                d = jb - 4 * ib
                for hl in (0, 1):
                    fulls.append((hl, jb, 0, 512, d))
            jd = 4 * ib
            # groups: list of (chunk, col_off).  Fulls are packed GCH
            # chunks to a psum group (column offsets 0/512) so one ACT
            # exp instruction covers up to 1024 columns.  Adjacent
            # offsets always land in different psum banks, so the
            # concurrently-running row-packed (h, h') score pairs never
            # write the same bank.  Diagonal cluster: group B holds the
            # d=2 pair (h@0, h'@512, strided exp skips the holes);
            # group A packs d=1 (384@0/512) and d=3 (128@384/896)
            # hole-free.  B is emitted before A so the d=3 chunk — the
            # accumulation-group stop — is the last PV.
            groups = []
            for i in range(0, len(fulls), GCH):
                groups.append([(c, j * 512) for j, c in enumerate(fulls[i : i + GCH])])
            groups.append([
                ((0, jd + 2, 256, 256, 2), 0),
                ((1, jd + 2, 256, 256, 2), 512),
            ])
            groups.append([
                ((0, jd + 1, 128, 384, 1), 0),
                ((1, jd + 1, 128, 384, 1), 512),
                ((0, jd + 3, 384, 128, 3), 384),
                ((1, jd + 3, 384, 128, 3), 896),
            ])

            po = [
                popool.tile([65, 512], f32, tag=f"po{hl}", name=f"po{pr}_{ib}_{hl}")
                for hl in (0, 1)
            ]
            for g in groups:
                extent = 512 if len(g) == 1 else 1024
                covered = sum(c[3] for c, _ in g)
                ps = psum.tile([128, extent], f32, tag="sc")
                for (hl, jb, qo, w, d), off in g:
                    nc.tensor.matmul(
                        ps[:, off : off + w],
                        kT[ts(hl, 64), ts(jb, 128)],
                        qT[ts(hl, 64), ib * 512 + qo : ib * 512 + qo + w],
                        start=True,
                        stop=True,
                    )
                pT = ppool.tile([128, extent], f16, tag="pT")
                if covered == extent:
                    nc.scalar.activation(pT[:], ps[:], EXP, scale=SM_SCALE)
                else:
                    # holed group (d=2 pair at 0/512): strided exp skips
                    # the stale psum columns (exp there could overflow)
                    w0 = g[0][0][3]
                    nc.scalar.activation(
                        pT[:].rearrange("p (q h) -> p q h", q=2)[:, :, 0:w0],
                        ps[:].rearrange("p (q h) -> p q h", q=2)[:, :, 0:w0],
                        EXP,
                        scale=SM_SCALE,
                    )
                # causal staircase on partial chunks; merge the
                # (h, h') twin segments (always 512 apart) into one 3-D op
                i = 0
                while i < len(g):
                    (hl, jb, qo, w, d), off_i = g[i]
                    if d < 0:
                        i += 1
                        continue
                    msk = stair_slice(d, qo, w)
                    twin = (
                        i + 1 < len(g)
                        and g[i + 1][0][1] == jb
                        and g[i + 1][0][3] == w
                        and g[i + 1][1] == off_i + 512
                    )
                    if twin:
                        seg = pT[:].rearrange("p (q h) -> p q h", q=2)[
                            :, :, off_i : off_i + w
                        ]
                        nc.vector.tensor_mul(
                            seg,
                            seg,
                            msk.unsqueeze(1).broadcast_to((128, 2, w)),
                        )
                        i += 2
                    else:
                        nc.vector.tensor_mul(
                            pT[:, off_i : off_i + w],
                            pT[:, off_i : off_i + w],
                            msk,
                        )
                        i += 1
                # PV: outT[d, i] accumulation per head; denominator
                # column (em) rides along as lhsT column 64.
                for (hl, jb, qo, w, d), off in g:
                    nc.tensor.matmul(
                        po[hl][:, qo : qo + w],
                        v2_sb[jb][:, (2 * pr + hl) * (HD + 1) : (2 * pr + hl + 1) * (HD + 1)],
                        pT[:, off : off + w],
                        start=(jb == 0),
                        stop=(jb == nch - 1),
                    )
            # normalize: out[:64] / denom (row 64), per column — both
            # heads batched through one reshape/reciprocal/broadcast
            # chain to halve the serial tail latency.
            sbp = opool.tile([65, 1024], f32, tag="sbp")
            for hl in (0, 1):
                nc.vector.tensor_copy(sbp[:, ts(hl, 512)], po[hl][:])
            rc = opool.tile([1, 1024], f32, tag="rc")
            if pr == 1 and ib == IBN - 1:
                # last call: ACT reciprocal straight off the denominator
                # row — saves the two serial reshape DMAs on the kernel
                # tail, where the ACT queue has nothing left to block
                act_reciprocal(rc[:], sbp[64:65, :])
            else:
                # reshape the denominator row onto 128 partitions so the
                # (slow, free-size-bound) DVE reciprocal runs on [128, 8]
                dn = opool.tile([128, 8], f32, tag="dn")
                nc.scalar.dma_start(
                    dn[:], sbp[64:65, :].rearrange("o (p c) -> o p c", p=128)
                )
                dnr = opool.tile([128, 8], f32, tag="dnr")
                nc.vector.reciprocal(dnr[:], dn[:])
                nc.sync.dma_start(
                    rc[:].rearrange("o (p c) -> o p c", p=128), dnr[:]
                )
            # broadcast/multiply/store per head so the first head's
            # output DMA overlaps the second head's broadcast
            rb = opool.tile([64, 1024], f32, tag="rb")
            oT = opool.tile([64, 1024], f32, tag="oT")
            for hl in (0, 1):
                nc.gpsimd.partition_broadcast(rb[:, ts(hl, 512)], rc[:, ts(hl, 512)])
                nc.vector.tensor_mul(
                    oT[:, ts(hl, 512)], sbp[0:64, ts(hl, 512)], rb[:, ts(hl, 512)]
                )
                nc.sync.dma_start(
                    outT[(2 * pr + hl) * HD : (2 * pr + hl + 1) * HD, ts(ib, 512)],
                    oT[:, ts(hl, 512)],
                )

        # emission order follows x-quarter arrival: per quarter, pair-0
        # projections + attention, then pair-1 likewise — projections
        # act as PE filler while ACT exps the other pair's scores.  The
        # wv fold is emitted after the first q/k projections so the PE
        # queue doesn't head-of-line block on the wv DMA.
        # Per quarter: ALL projections first (their DVE bias-adds and v2
        # ops precede the attention masks in the in-order DVE queue, so
        # weight production is never head-of-line blocked behind masks),
        # then the two attention calls.
        project_qk(wq_f, qT_sb[0], bias_sb[:, 0:1], 0, 0)
        project_qk(wk_all, kT_sb[0], bias_sb[:, 2:3], 0, 0)
        wv_f = load_folded(wv_all, DM, DPC, "v")
        for nb in range(NB):
            if nb > 0:
                project_qk(wq_f, qT_sb[0], bias_sb[:, 0:1], 0, nb)
                project_qk(wk_all, kT_sb[0], bias_sb[:, 2:3], 0, nb)
            project_v(range(4 * nb, 4 * nb + 4))
            project_qk(wq_f, qT_sb[1], bias_sb[:, 1:2], 1, nb)
            project_qk(wk_all, kT_sb[1], bias_sb[:, 3:4], 1, nb)
            attention_ib(0, nb)
            attention_ib(1, nb)

    nc.compile()
    return nc


def _shard_inputs(inputs):
    """Full inputs -> per-core input maps (host-side layout + fp16 cast)."""
    f16 = np.float16
    hs = np.asarray(inputs["hidden_states"], dtype=np.float32)
    am = np.asarray(inputs["attention_mask"], dtype=np.float32)
    Wq = np.asarray(inputs["Wq"], dtype=np.float32)
    Wk = np.asarray(inputs["Wk"], dtype=np.float32)
    Wv = np.asarray(inputs["Wv"], dtype=np.float32)
    bq = np.asarray(inputs["bq"], dtype=np.float32)
    bk = np.asarray(inputs["bk"], dtype=np.float32)
    bv = np.asarray(inputs["bv"], dtype=np.float32)
    Aq = np.asarray(inputs["Aq"], dtype=np.float32)
    Bq = np.asarray(inputs["Bq"], dtype=np.float32)
    Av = np.asarray(inputs["Av"], dtype=np.float32)
    Bv = np.asarray(inputs["Bv"], dtype=np.float32)

    c = np.ascontiguousarray

    def wimg(W, sl):
        # sbuf image: wimg[p, 256*kc + j] = W[sl].T[128*kc + p, j]
        return c(W[sl].T.astype(f16).reshape(KC, 128, DPC).transpose(1, 0, 2).reshape(128, KC * DPC))

    # x quarter images: Xq[q, p, 512*kc + cc] = hs[b].T[128*kc + p, 512*q + cc]
    xqs = [
        c(hs[b].T.astype(f16).reshape(KC, 128, NB, 512).transpose(2, 1, 0, 3).reshape(NB, 128, KC * 512))
        for b in range(B)
    ]
    a_both = c(np.concatenate([Aq, Av], axis=1).astype(f16))
    in_maps = []
    for core in range(NCORES):
        b, g = core // GPB, core % GPB
        sl = slice(g * DPC, (g + 1) * DPC)
        bqk = np.stack(
            [bq[sl][:128], bq[sl][128:], bk[sl][:128], bk[sl][128:]], axis=1
        )
        bT = np.concatenate(
            [LORA_SCALE * Bq[sl].T, LORA_SCALE * Bv[sl].T], axis=1
        ).astype(f16)
        in_maps.append(
            {
                "xq": xqs[b],
                "wq_img": wimg(Wq, sl),
                "wk_img": wimg(Wk, sl),
                "wv_img": wimg(Wv, sl),
                "a_both": a_both,
                "bT_both": c(bT),
                "biasqk": c(bqk),
                "biasv": c(bv[sl].astype(f16)),
                "amask": c(am[b, 0, 0, :]),
            }
        )
    return in_maps


def _run(inputs, trace=False):
    from concourse.bass_utils import run_bass_kernel_spmd

    if "nc" not in _CACHE:
        _CACHE["nc"] = _build_program()
    nc = _CACHE["nc"]
    in_maps = _shard_inputs(inputs)
    res = run_bass_kernel_spmd(nc, in_maps, list(range(NCORES)), trace=trace)
    out = np.empty((B, T, DM), dtype=np.float32)
    for core in range(NCORES):
        b, g = core // GPB, core % GPB
        out[b, :, g * DPC : (g + 1) * DPC] = res.results[core]["outT"].T
    return out, res


def kernel(**inputs) -> np.ndarray:
    out, _ = _run(inputs, trace=False)
    return out


# revision 39
# speedup vs baseline: 1.0655x; 1.0130x over previous
"""Causal self-attention with LoRA (q,v) — Trainium2 Bass kernel, 8 cores.

Sharding: data-parallel over batch (B=2), tensor-parallel over heads
(16 heads -> 4 per core).  Core c handles batch c//4, heads 4*(c%4)..+4.
Each core computes its 256-dim q/k/v projection slice from the full
hidden states and its heads' full 2048x2048 causal attention locally.
No collectives; host does the (layout-only) scatter/gather + fp16 casts
+ sbuf-image tiling so every bulk load is one contiguous HWDGE DMA.

The whole datapath runs in fp16 with fp32 PSUM accumulation (measured
rel err vs the fp32 reference ~5e-4).  fp16 matters a lot on TRN2: f32r
matmuls cannot use standalone LDWEIGHTS (walrus bug) so every f32r MM
pays an inline ~214ns weight load, and FP32_HIGH disables fast weight
load entirely; fp16 streams at 1 col/cycle with LDW hidden by the PE
reorder window.

Attention is computed in transposed orientation (scores sT[j, i]) so no
on-chip transposes are needed; the softmax denominator rides along the
PV matmul as a 65th lhsT column (augmented-V), which also folds the
additive attention mask in exactly (em = exp(mask) scaling of V rows).
Score psum groups pack 3 head-chunks (1536 cols) so the ACT exp — the
throughput limiter of the attention phase at 1 elem/lane/cycle — pays
its ~352-cycle per-instruction overhead a third as often.  Causal
masking is a multiplicative staircase applied after exp; the two
most-diagonal 128-key chunks per 512-query block are cropped to their
upper 256 columns (laid out so concurrently issued row-packed matmul
pairs never write the same PSUM bank).  Scores for the two heads of a
pair run concurrently via K=64 row packing.  Emission follows x-quarter
arrival so the first attention block starts ~10us in.
"""

import numpy as np

B, T, DM, H = 2, 2048, 1024, 16
HD = 64          # head dim
R = 8            # LoRA rank
NCORES = 8
GPB = 4          # head-groups (cores) per batch
HPC = 4          # heads per core
DPC = HPC * HD   # 256 output dims per core
LORA_SCALE = 2.0  # lora_alpha / r
SM_SCALE = HD ** -0.5  # 0.125

KC = DM // 128   # 8 contraction chunks
MC = DPC // 128  # 2 output-dim chunks (head pairs)
NB = 4           # t-blocks (x quarters) for q/k projections
TCH = T // 128   # 16 t-chunks (key chunks)
IBN = T // 512   # 4 query i-blocks (512 wide)
GCH = 2          # score-group capacity in 512-wide chunk units

_CACHE = {}


def _build_program():
    from contextlib import ExitStack

    import concourse.bass as bass
    import concourse.tile as tile
    from concourse import bacc, mybir

    f32 = mybir.dt.float32
    f16 = mybir.dt.float16
    EXP = mybir.ActivationFunctionType.Exp
    ts = bass.ts

    nc = bacc.Bacc(
        "TRN2",
        target_bir_lowering=False,
        debug=False,
        enable_asserts=True,
        num_devices=NCORES,
    )

    xq = nc.dram_tensor("xq", [NB, 128, KC * 512], f16, kind="ExternalInput").ap()
    wq_img = nc.dram_tensor("wq_img", [128, KC * DPC], f16, kind="ExternalInput").ap()
    wk_img = nc.dram_tensor("wk_img", [128, KC * DPC], f16, kind="ExternalInput").ap()
    wv_img = nc.dram_tensor("wv_img", [128, KC * DPC], f16, kind="ExternalInput").ap()
    a_both = nc.dram_tensor("a_both", [R, 2 * DM], f16, kind="ExternalInput").ap()
    bT_both = nc.dram_tensor("bT_both", [R, 2 * DPC], f16, kind="ExternalInput").ap()
    biasqk = nc.dram_tensor("biasqk", [128, 4], f32, kind="ExternalInput").ap()
    biasv = nc.dram_tensor("biasv", [DPC], f16, kind="ExternalInput").ap()
    amask = nc.dram_tensor("amask", [T], f32, kind="ExternalInput").ap()
    outT = nc.dram_tensor("outT", [DPC, T], f32, kind="ExternalOutput").ap()

    with tile.TileContext(nc) as tc, ExitStack() as ctx:
        const = ctx.enter_context(tc.tile_pool(name="const", bufs=1))
        xpool = ctx.enter_context(tc.tile_pool(name="x", bufs=1))
        wpool = ctx.enter_context(tc.tile_pool(name="w", bufs=1))
        qkpool = ctx.enter_context(tc.tile_pool(name="qk", bufs=1))
        vpool = ctx.enter_context(tc.tile_pool(name="v", bufs=1))
        ppool = ctx.enter_context(tc.tile_pool(name="pT", bufs=4))
        opool = ctx.enter_context(tc.tile_pool(name="osb", bufs=4))
        psum = ctx.enter_context(tc.tile_pool(name="psum", bufs=3, space="PSUM"))
        popool = ctx.enter_context(tc.tile_pool(name="po", bufs=1, space="PSUM"))

        def stair_slice(d, qo, w):
            # chunk with diagonal offset d cropped to queries [qo, qo+w):
            # valid iff j >= p + 128 d - qo; stair[p, s+j] = (j >= p) at
            # s = 384, shifted by the residue 128 d - qo (0 for every
            # crop in use, since qo = 128 d).
            start = 384 - 128 * d + qo
            return stair[:, start : start + w]

        # ---------------- loads, critical-path first ------------------
        # DMA engines drain queue descriptors roughly FIFO across
        # queues, so the small LoRA/weight transfers the first matmuls
        # depend on are issued before the 4MB of x.
        # wq first: the tensor queue's merged startup wait releases at
        # the LAST of {a, bT, wq}, and wq is the biggest of the three
        wq_all = wpool.tile([128, KC * DPC], f16, tag="wq_all")
        nc.sync.dma_start(wq_all[:], wq_img)
        a_sb = const.tile([R, 2 * DM], f16, tag="a")
        nc.sync.dma_start(a_sb[:], a_both)
        bT_sb = const.tile([R, 2 * DPC], f16, tag="bT")
        nc.sync.dma_start(bT_sb[:], bT_both)
        wk_all = wpool.tile([128, KC * DPC], f16, tag="wk_all")
        nc.sync.dma_start(wk_all[:], wk_img)

        xall = xpool.tile([128, KC * T], f16, tag="xall")
        x3d = xall[:].rearrange("p (kc c) -> p kc c", kc=KC)
        # first quarter split in two so the first projection matmuls
        # (kc 0-3) can start after 512KB instead of 1MB
        nc.scalar.dma_start(x3d[:, 0:4, ts(0, 512)], xq[0][:, 0 : 4 * 512])
        nc.scalar.dma_start(x3d[:, 4:8, ts(0, 512)], xq[0][:, 4 * 512 : 8 * 512])

        wv_all = wpool.tile([128, KC * DPC], f16, tag="wv_all")
        nc.sync.dma_start(wv_all[:], wv_img)
        bias_sb = const.tile([128, 4], f32, tag="biasqk")
        nc.sync.dma_start(bias_sb[:], biasqk)
        bv_row = const.tile([1, DPC], f16, tag="bvrow")
        nc.sync.dma_start(bv_row[:], biasv.unsqueeze(0))
        em_raw = const.tile([128, TCH], f32, tag="em_raw")
        nc.sync.dma_start(em_raw[:], amask.rearrange("(c p) -> p c", p=128))

        for q in range(1, NB):
            nc.scalar.dma_start(x3d[:, :, ts(q, 512)], xq[q])

        def xs(kc, lo, n):
            return xall[:, 2048 * kc + lo : 2048 * kc + lo + n]

        ones_1xP = const.tile([1, 128], f16, tag="ones")
        nc.vector.memset(ones_1xP[:], 1.0)

        # dummy matmuls fill the PE while the first DMAs land, warming
        # the HAM clock-gate (first ~3.4us of PE activity runs at 1.2
        # instead of 2.4 GHz) so the real stream starts at full clock
        for _ in range(24):
            wps = psum.tile([128, 128], f32, tag="sc")
            nc.tensor.matmul(wps[:], ones_1xP[:], ones_1xP[:], start=True, stop=True)

        # em[p, jb] = exp(amask[128*jb + p])
        em = const.tile([128, TCH], f32, tag="em")
        nc.scalar.activation(em[:], em_raw[:], EXP)

        # Causal staircase (multiplicative, applied after exp):
        # stair[p, m] = 1.0 if m >= p + 384 else 0.0 ; shape [128, 896].
        stair = const.tile([128, 896], f16, tag="stair")
        nc.gpsimd.memset(stair[:], 1.0)
        nc.gpsimd.affine_select(
            out=stair[:],
            in_=stair[:],
            compare_op=mybir.AluOpType.is_ge,
            fill=0.0,
            base=-384,
            pattern=[[1, 896]],
            channel_multiplier=-1,
        )

        # ---------------- weights with LoRA fold for q, v ---------------
        def load_folded(w_all, a_off, bT_off, name):
            """W'.T = W.T + A.T @ (2 B.T), one [128, KC*DPC] tile."""
            wf = wpool.tile([128, KC * DPC], f16, tag=f"wf_{name}")
            for kc in range(KC):
                dps = psum.tile([128, DPC], f32, tag="sc")
                nc.tensor.matmul(
                    dps[:],
                    a_sb[:, a_off + 128 * kc : a_off + 128 * kc + 128],
                    bT_sb[:, bT_off : bT_off + DPC],
                    start=True,
                    stop=True,
                )
                nc.vector.tensor_add(
                    wf[:, ts(kc, DPC)], w_all[:, ts(kc, DPC)], dps[:]
                )
            return wf

        wq_f = load_folded(wq_all, 0, 0, "q")

        # ---------------- projections ----------------
        # qT/kT: [d, t] with d on partitions; tile mc holds head pair
        # (2mc, 2mc+1): partitions 0-63 = head 2mc, 64-127 = head 2mc+1.
        qT_sb = [qkpool.tile([128, T], f16, tag=f"qT{mc}", name=f"qT{mc}") for mc in range(MC)]
        kT_sb = [qkpool.tile([128, T], f16, tag=f"kT{mc}", name=f"kT{mc}") for mc in range(MC)]

        def project_qk(wf, dst, bias, mc, nb):
            ps = psum.tile([128, 512], f32, tag="sc")
            for kc in range(KC):
                nc.tensor.matmul(
                    ps[:],
                    wf[:, kc * DPC + mc * 128 : kc * DPC + mc * 128 + 128],
                    xs(kc, 512 * nb, 512),
                    start=(kc == 0),
                    stop=(kc == KC - 1),
                )
            nc.vector.tensor_add(
                dst[:, ts(nb, 512)],
                ps[:],
                bias.to_broadcast((128, 512)),
            )

        # v in natural [t, d] orientation, em-scaled, with the denominator
        # (em) column appended per head: [128, 4*65].
        v2_sb = [vpool.tile([128, HPC * (HD + 1)], f16, tag=f"v2{j}", name=f"v2{j}") for j in range(TCH)]

        def project_v(jbs):
            for jb in jbs:
                ps = psum.tile([128, DPC], f32, tag="sc")
                for kc in range(KC):
                    nc.tensor.matmul(
                        ps[:],
                        xs(kc, 128 * jb, 128),
                        wv_f[:, ts(kc, DPC)],
                        start=(kc == 0),
                        stop=False,
                    )
                nc.tensor.matmul(  # + ones(t) x bias_v
                    ps[:],
                    ones_1xP[:],
                    bv_row[:],
                    start=False,
                    stop=True,
                )
                v2 = v2_sb[jb]
                em_col = em[:, jb : jb + 1]
                nc.vector.tensor_mul(
                    v2[:].rearrange("p (h c) -> p h c", h=HPC)[:, :, 0:HD],
                    ps[:].rearrange("p (h c) -> p h c", h=HPC),
                    em_col.unsqueeze(1).broadcast_to((128, HPC, HD)),
                )
                nc.vector.tensor_copy(
                    v2[:, HD : HPC * (HD + 1) : HD + 1],
                    em_col.to_broadcast((128, HPC)),
                )

        def act_reciprocal(out, in_):
            # raw InstActivation: bass's activation() refuses Reciprocal
            # (spline accuracy ~1e-3 rel), fine for the final 1/denom
            # given the ~40x error margin; used only on the last call's
            # tail where it replaces two serial sbuf-reshape DMAs.
            eng = nc.scalar
            ins = [eng.lower_ap(in_)]
            for v in (0.0, 1.0, 0.0):
                ins.append(mybir.ImmediateValue(dtype=mybir.dt.float32, value=v))
            return eng.add_instruction(
                mybir.InstActivation(
                    name=nc.get_next_instruction_name(),
                    func=mybir.ActivationFunctionType.Reciprocal,
                    ins=ins,
                    outs=[eng.lower_ap(out)],
                )
            )

        # ---------------- attention for one head pair ----------------
        def attention_ib(pr, ib):
            qT, kT = qT_sb[pr], kT_sb[pr]
            nch = 4 * ib + 4  # causal key chunks per head
            # chunk stream: (hl, jb, qoff, width, d); d = diagonal
            # offset.  Diagonal chunks are cropped to exactly the
            # causally reachable query range: d=1 -> 384@128, d=2 ->
            # 256@256, d=3 -> 128@384 (d=0 stays full so the first PV
            # write covers the whole po region).
            fulls = []
            for jb in range(4 * ib + 1):
                d = jb - 4 * ib
                for hl in (0, 1):
                    fulls.append((hl, jb, 0, 512, d))
            jd = 4 * ib
            # groups: list of (chunk, col_off).  Fulls are packed GCH
            # chunks to a psum group (column offsets 0/512) so one ACT
            # exp instruction covers up to 1024 columns.  Adjacent
            # offsets always land in different psum banks, so the
            # concurrently-running row-packed (h, h') score pairs never
            # write the same bank.  Diagonal cluster: group B holds the
            # d=2 pair (h@0, h'@512, strided exp skips the holes);
            # group A packs d=1 (384@0/512) and d=3 (128@384/896)
            # hole-free.  B is emitted before A so the d=3 chunk — the
            # accumulation-group stop — is the last PV.
            groups = []
            for i in range(0, len(fulls), GCH):
                groups.append([(c, j * 512) for j, c in enumerate(fulls[i : i + GCH])])
            groups.append([
                ((0, jd + 2, 256, 256, 2), 0),
                ((1, jd + 2, 256, 256, 2), 512),
            ])
            groups.append([
                ((0, jd + 1, 128, 384, 1), 0),
                ((1, jd + 1, 128, 384, 1), 512),
                ((0, jd + 3, 384, 128, 3), 384),
                ((1, jd + 3, 384, 128, 3), 896),
            ])

            po = [
                popool.tile([65, 512], f32, tag=f"po{hl}", name=f"po{pr}_{ib}_{hl}")
                for hl in (0, 1)
            ]
            for g in groups:
                extent = 512 if len(g) == 1 else 1024
                covered = sum(c[3] for c, _ in g)
                ps = psum.tile([128, extent], f32, tag="sc")
                for (hl, jb, qo, w, d), off in g:
                    nc.tensor.matmul(
                        ps[:, off : off + w],
                        kT[ts(hl, 64), ts(jb, 128)],
                        qT[ts(hl, 64), ib * 512 + qo : ib * 512 + qo + w],
                        start=True,
                        stop=True,
                    )
                pT = ppool.tile([128, extent], f16, tag="pT")
                if covered == extent:
                    nc.scalar.activation(pT[:], ps[:], EXP, scale=SM_SCALE)
                else:
                    # holed group (d=2 pair at 0/512): strided exp skips
                    # the stale psum columns (exp there could overflow)
                    w0 = g[0][0][3]
                    nc.scalar.activation(
                        pT[:].rearrange("p (q h) -> p q h", q=2)[:, :, 0:w0],
                        ps[:].rearrange("p (q h) -> p q h", q=2)[:, :, 0:w0],
                        EXP,
                        scale=SM_SCALE,
                    )
                # causal staircase on partial chunks; merge the
                # (h, h') twin segments (always 512 apart) into one 3-D op
                i = 0
                while i < len(g):
                    (hl, jb, qo, w, d), off_i = g[i]
                    if d < 0:
                        i += 1
                        continue
                    msk = stair_slice(d, qo, w)
                    twin = (
                        i + 1 < len(g)
                        and g[i + 1][0][1] == jb
                        and g[i + 1][0][3] == w
                        and g[i + 1][1] == off_i + 512
                    )
                    if twin:
                        seg = pT[:].rearrange("p (q h) -> p q h", q=2)[
                            :, :, off_i : off_i + w
                        ]
                        nc.vector.tensor_mul(
                            seg,
                            seg,
                            msk.unsqueeze(1).broadcast_to((128, 2, w)),
                        )
                        i += 2
                    else:
                        nc.vector.tensor_mul(
                            pT[:, off_i : off_i + w],
                            pT[:, off_i : off_i + w],
                            msk,
                        )
                        i += 1
                # PV: outT[d, i] accumulation per head; denominator
                # column (em) rides along as lhsT column 64.
                for (hl, jb, qo, w, d), off in g:
                    nc.tensor.matmul(
                        po[hl][:, qo : qo + w],
                        v2_sb[jb][:, (2 * pr + hl) * (HD + 1) : (2 * pr + hl + 1) * (HD + 1)],
                        pT[:, off : off + w],
                        start=(jb == 0),
                        stop=(jb == nch - 1),
                    )
            # normalize: out[:64] / denom (row 64), per column — both
            # heads batched through one reshape/reciprocal/broadcast
            # chain to halve the serial tail latency.
            sbp = opool.tile([65, 1024], f32, tag="sbp")
            rc = opool.tile([1, 1024], f32, tag="rc")
            if pr == 1 and ib == IBN - 1:
                # last call: denominator rows copied first, then ACT
                # reciprocal straight off the row (replacing the two
                # serial reshape DMAs) while the bulk rows still copy —
                # the ACT queue has nothing left to block on the tail
                for hl in (0, 1):
                    nc.vector.tensor_copy(sbp[64:65, ts(hl, 512)], po[hl][64:65, :])
                act_reciprocal(rc[:], sbp[64:65, :])
                for hl in (0, 1):
                    nc.vector.tensor_copy(sbp[0:64, ts(hl, 512)], po[hl][0:64, :])
            else:
                for hl in (0, 1):
                    nc.vector.tensor_copy(sbp[:, ts(hl, 512)], po[hl][:])
                # reshape the denominator row onto 128 partitions so the
                # (slow, free-size-bound) DVE reciprocal runs on [128, 8]
                dn = opool.tile([128, 8], f32, tag="dn")
                nc.scalar.dma_start(
                    dn[:], sbp[64:65, :].rearrange("o (p c) -> o p c", p=128)
                )
                dnr = opool.tile([128, 8], f32, tag="dnr")
                nc.vector.reciprocal(dnr[:], dn[:])
                nc.sync.dma_start(
                    rc[:].rearrange("o (p c) -> o p c", p=128), dnr[:]
                )
            # broadcast/multiply/store per head so the first head's
            # output DMA overlaps the second head's broadcast
            rb = opool.tile([64, 1024], f32, tag="rb")
            oT = opool.tile([64, 1024], f32, tag="oT")
            for hl in (0, 1):
                nc.gpsimd.partition_broadcast(rb[:, ts(hl, 512)], rc[:, ts(hl, 512)])
                nc.vector.tensor_mul(
                    oT[:, ts(hl, 512)], sbp[0:64, ts(hl, 512)], rb[:, ts(hl, 512)]
                )
                nc.sync.dma_start(
                    outT[(2 * pr + hl) * HD : (2 * pr + hl + 1) * HD, ts(ib, 512)],
                    oT[:, ts(hl, 512)],
                )

        # emission order follows x-quarter arrival: per quarter, pair-0
        # projections + attention, then pair-1 likewise — projections
        # act as PE filler while ACT exps the other pair's scores.  The
        # wv fold is emitted after the first q/k projections so the PE
        # queue doesn't head-of-line block on the wv DMA.
        # Per quarter: ALL projections first (their DVE bias-adds and v2
        # ops precede the attention masks in the in-order DVE queue, so
        # weight production is never head-of-line blocked behind masks),
        # then the two attention calls.
        project_qk(wq_f, qT_sb[0], bias_sb[:, 0:1], 0, 0)
        project_qk(wk_all, kT_sb[0], bias_sb[:, 2:3], 0, 0)
        wv_f = load_folded(wv_all, DM, DPC, "v")
        for nb in range(NB):
            if nb > 0:
                project_qk(wq_f, qT_sb[0], bias_sb[:, 0:1], 0, nb)
                project_qk(wk_all, kT_sb[0], bias_sb[:, 2:3], 0, nb)
            project_v(range(4 * nb, 4 * nb + 4))
            project_qk(wq_f, qT_sb[1], bias_sb[:, 1:2], 1, nb)
            project_qk(wk_all, kT_sb[1], bias_sb[:, 3:4], 1, nb)
            attention_ib(0, nb)
            attention_ib(1, nb)

    nc.compile()
    return nc


def _shard_inputs(inputs):
    """Full inputs -> per-core input maps (host-side layout + fp16 cast)."""
    f16 = np.float16
    hs = np.asarray(inputs["hidden_states"], dtype=np.float32)
    am = np.asarray(inputs["attention_mask"], dtype=np.float32)
    Wq = np.asarray(inputs["Wq"], dtype=np.float32)
    Wk = np.asarray(inputs["Wk"], dtype=np.float32)
    Wv = np.asarray(inputs["Wv"], dtype=np.float32)
    bq = np.asarray(inputs["bq"], dtype=np.float32)
    bk = np.asarray(inputs["bk"], dtype=np.float32)
    bv = np.asarray(inputs["bv"], dtype=np.float32)
    Aq = np.asarray(inputs["Aq"], dtype=np.float32)
    Bq = np.asarray(inputs["Bq"], dtype=np.float32)
    Av = np.asarray(inputs["Av"], dtype=np.float32)
    Bv = np.asarray(inputs["Bv"], dtype=np.float32)

    c = np.ascontiguousarray

    def wimg(W, sl):
        # sbuf image: wimg[p, 256*kc + j] = W[sl].T[128*kc + p, j]
        return c(W[sl].T.astype(f16).reshape(KC, 128, DPC).transpose(1, 0, 2).reshape(128, KC * DPC))

    # x quarter images: Xq[q, p, 512*kc + cc] = hs[b].T[128*kc + p, 512*q + cc]
    xqs = [
        c(hs[b].T.astype(f16).reshape(KC, 128, NB, 512).transpose(2, 1, 0, 3).reshape(NB, 128, KC * 512))
        for b in range(B)
    ]
    a_both = c(np.concatenate([Aq, Av], axis=1).astype(f16))
    in_maps = []
    for core in range(NCORES):
        b, g = core // GPB, core % GPB
        sl = slice(g * DPC, (g + 1) * DPC)
        bqk = np.stack(
            [bq[sl][:128], bq[sl][128:], bk[sl][:128], bk[sl][128:]], axis=1
        )
        bT = np.concatenate(
            [LORA_SCALE * Bq[sl].T, LORA_SCALE * Bv[sl].T], axis=1
        ).astype(f16)
        in_maps.append(
            {
                "xq": xqs[b],
                "wq_img": wimg(Wq, sl),
                "wk_img": wimg(Wk, sl),
                "wv_img": wimg(Wv, sl),
                "a_both": a_both,
                "bT_both": c(bT),
                "biasqk": c(bqk),
                "biasv": c(bv[sl].astype(f16)),
                "amask": c(am[b, 0, 0, :]),
            }
        )
    return in_maps


def _run(inputs, trace=False):
    from concourse.bass_utils import run_bass_kernel_spmd

    if "nc" not in _CACHE:
        _CACHE["nc"] = _build_program()
    nc = _CACHE["nc"]
    in_maps = _shard_inputs(inputs)
    res = run_bass_kernel_spmd(nc, in_maps, list(range(NCORES)), trace=trace)
    out = np.empty((B, T, DM), dtype=np.float32)
    for core in range(NCORES):
        b, g = core // GPB, core % GPB
        out[b, :, g * DPC : (g + 1) * DPC] = res.results[core]["outT"].T
    return out, res


def kernel(**inputs) -> np.ndarray:
    out, _ = _run(inputs, trace=False)
    return out
